# revision 1
# baseline (speedup 1.0000x reference)
"""DetectionLoss Trainium2 kernel.

Contract: kernel(**inputs) takes FULL inputs (bbox_pred [16,65536,4],
conf_pred [16,65536], anchors [65536,4], target_boxes [16,32,4]) and
returns the full output (total_loss, conf_loss, bbox_loss) as f32 scalars.

Sharding: data-parallel over batch. Core k processes images 2k, 2k+1 and
emits (sum conf_l, sum bbox_l) over its two images; the host divides by 16
and sums conf+bbox for the total.

Per-image algorithm (all IoU uses are monotone compares, so we work in
log space and never divide in the hot loop):
  packed[a,t] = ln(inter+1e-38) + (31-t)*2^-20 - ln(union+EPS)
  - mp[a] = max_t packed  -> pos/neg via threshold compares (log consts)
  - per-target top-3 via the DVE top-8 instruction (exact two-level merge)
  - forced positives: packed >= thr_t accumulated with a fused is_ge/max
  - matched box: exact equality packed==mp selects argmax target (index
    perturbation reproduces jnp.argmax first-index tie-break)
  - hard-negative top-k sum via binary search on the focal threshold
"""

import os
from contextlib import ExitStack

import numpy as np

P = 128          # SBUF partitions
F = 512          # anchors per partition row
T = 32           # targets per image
BI = 2           # images per core
A = P * F        # 65536 anchors
NCORES = 8

EPS = 1e-6
TINY = 1e-38
EPSI = float(2.0 ** -20)          # index-packing epsilon (log space)
DELTA = 2e-6                      # strictness margin for iou > 0.3
LN05 = float(np.log(0.5)) + 15.0 * EPSI
LN04 = float(np.log(0.4)) + 15.0 * EPSI
LN03D = float(np.log(0.3)) + DELTA
N_SEARCH = 13                     # binary-search iterations for kth value


def _emit(nc, tc, ctx, bp_d, cp_d, an_d, tb_d, out_d, sc1_d, sc2_d, sc3_d, sc4_d):
    import concourse.bass as bass
    import concourse.mybir as mybir

    f32 = mybir.dt.float32
    i32 = mybir.dt.int32
    Alu = mybir.AluOpType
    Act = mybir.ActivationFunctionType
    V = nc.vector
    S = nc.scalar
    G = nc.gpsimd
    PE = nc.tensor
    ts = bass.ts

    big = ctx.enter_context(tc.tile_pool(name="big", bufs=1))
    stage = ctx.enter_context(tc.tile_pool(name="stage", bufs=1))
    per = ctx.enter_context(tc.tile_pool(name="per", bufs=1))
    tp = ctx.enter_context(tc.tile_pool(name="tp", bufs=2))
    sm = ctx.enter_context(tc.tile_pool(name="sm", bufs=2))
    ps = ctx.enter_context(tc.tile_pool(name="ps", bufs=2, space="PSUM"))

    # ---- setup: anchors ----
    AXYS = stage.tile([P, F * 4], f32, tag="AXYS")
    nc.gpsimd.dma_start(AXYS[:], an_d.rearrange("(p f) c -> p (f c)", p=P))
    AXv = AXYS[:].rearrange("p (f c) -> p c f", c=4)
    AX1 = big.tile([P, F], f32)
    AY1 = big.tile([P, F], f32)
    AX2 = big.tile([P, F], f32)
    AY2 = big.tile([P, F], f32)
    V.tensor_copy(AX1[:], AXv[:, 0])
    V.tensor_copy(AY1[:], AXv[:, 1])
    V.tensor_copy(AX2[:], AXv[:, 2])
    V.tensor_copy(AY2[:], AXv[:, 3])
    AREAA = big.tile([P, F], f32)
    aw0 = tp.tile([P, F], f32, tag="uxy")
    ah0 = tp.tile([P, F], f32, tag="dx")
    V.tensor_sub(aw0[:], AX2[:], AX1[:])
    V.tensor_sub(ah0[:], AY2[:], AY1[:])
    V.tensor_mul(AREAA[:], aw0[:], ah0[:])

    # (31 - t) * EPSI row, same on all partitions
    KEPS = big.tile([P, T], f32)
    for t in range(T):
        V.memset(KEPS[:, t : t + 1], (31 - t) * EPSI)

    # per-target ln(0.3) + (31-t)*EPSI + DELTA  (on partitions 0..31)
    L03row = sm.tile([1, T], f32)
    for t in range(T):
        V.memset(L03row[:, t : t + 1], LN03D + (31 - t) * EPSI)
    nc.gpsimd.dma_start(sc3_d, L03row[:])
    L03C = big.tile([T, 1], f32)
    nc.gpsimd.dma_start(L03C[:], sc3_d.rearrange("(t one) -> t one", one=1))

    ONES = big.tile([P, 1], f32)
    V.memset(ONES[:], 1.0)
    ONESROW = big.tile([1, P], f32)
    V.memset(ONESROW[:], 1.0)
    # identity matrix for PE transposes, built via DRAM bounce:
    # flat positions k*129 in a [128,129] grid form the diagonal of [128,128]
    ZED = stage.tile([P, P], f32, tag="BPS", bufs=2, name="ZED")
    V.memset(ZED[:], 0.0)
    nc.gpsimd.dma_start(sc4_d[: P * P].rearrange("(p f) -> p f", p=P), ZED[:])
    nc.gpsimd.dma_start(
        sc4_d[: P * 129].rearrange("(p f) -> p f", f=129)[:, 0:1], ONES[:])
    IDTV = big.tile([P, P], f32)
    nc.gpsimd.dma_start(IDTV[:], sc4_d[: P * P].rearrange("(p f) -> p f", p=P))
    IDTB = big.tile([P, P], mybir.dt.bfloat16)
    V.tensor_copy(IDTB[:], IDTV[:])
    CTINY = big.tile([P, 1], f32)
    V.memset(CTINY[:], TINY)
    OUT = big.tile([1, 2], f32)
    V.memset(OUT[:], 0.0)

    IOU = big.tile([P, T * F], f32)     # packed log-iou, t-major slices

    for b in range(BI):
        # ---- per-image loads ----
        BPS = stage.tile([P, F * 4], f32, tag="BPS", bufs=2)
        nc.gpsimd.dma_start(BPS[:], bp_d[b].rearrange("(p f) c -> p (f c)", p=P))
        BPv = BPS[:].rearrange("p (f c) -> p c f", c=4)
        BX1 = per.tile([P, F], f32, tag="BX1")
        BY1 = per.tile([P, F], f32, tag="BY1")
        BX2 = per.tile([P, F], f32, tag="BX2")
        BY2 = per.tile([P, F], f32, tag="BY2")
        V.tensor_copy(BX1[:], BPv[:, 0])
        V.tensor_copy(BY1[:], BPv[:, 1])
        V.tensor_copy(BX2[:], BPv[:, 2])
        V.tensor_copy(BY2[:], BPv[:, 3])
        CPt = per.tile([P, F], f32, tag="CPt", bufs=2)
        nc.gpsimd.dma_start(CPt[:], cp_d[b].rearrange("(p f) -> p f", p=P))

        # block-diagonal tb: rows 32f'..32f'+32 have tb in cols 4f'..4f'+4
        TBD = sm.tile([P, 16], f32, bufs=2)
        V.memset(TBD[:], 0.0)
        for rep in range(4):
            nc.gpsimd.dma_start(
                TBD[T * rep : T * (rep + 1), 4 * rep : 4 * rep + 4], tb_d[b])
        TBDV = sm.tile([P, 16], f32, bufs=2)
        S.copy(TBDV[:], TBD[:])
        TBrow = sm.tile([1, T * 4], f32, bufs=2)
        nc.gpsimd.dma_start(TBrow[:], tb_d[b].rearrange("t c -> (t c)")[None, :])
        TBrowV = sm.tile([1, T * 4], f32, bufs=2)
        V.tensor_copy(TBrowV[:], TBrow[:])
        TBB = per.tile([P, T * 4], f32, tag="TBB")
        tbb_ps = ps.tile([P, T * 4], f32, tag="bc_ps", bufs=1, name="tbb_ps")
        PE.matmul(tbb_ps[:], ONESROW[:], TBrowV[:], start=True, stop=True)
        V.tensor_copy(TBB[:], tbb_ps[:])
        TBv = TBB[:].rearrange("p (t c) -> p c t", c=4)
        TW = sm.tile([P, T], f32)
        TH = sm.tile([P, T], f32)
        ABE = per.tile([P, T], f32, tag="ABE")
        V.tensor_sub(TW[:], TBv[:, 2], TBv[:, 0])
        V.tensor_sub(TH[:], TBv[:, 3], TBv[:, 1])
        V.tensor_mul(ABE[:], TW[:], TH[:])
        V.tensor_scalar(ABE[:], ABE[:], EPS, None, op0=Alu.add)

        def tcol(tt, c):
            return TBB[:, 4 * tt + c : 4 * tt + c + 1]

        mp = per.tile([P, F], f32, tag="mp")
        RM8 = per.tile([P, T * 8], f32, tag="RM8")
        IOUtm = IOU[:].rearrange("p (f t) -> p t f", t=T)
        # ---- pass 1: packed log-iou (t-minor layout: free = f*32 + t) ----
        for t in range(T):
            ux = tp.tile([P, F], f32, tag="uxy")
            V.tensor_scalar(ux[:], AX1[:], tcol(t, 0), None, op0=Alu.max)
            dx = tp.tile([P, F], f32, tag="dx")
            V.scalar_tensor_tensor(dx[:], AX2[:], tcol(t, 2), ux[:],
                                   op0=Alu.min, op1=Alu.subtract)
            uy = tp.tile([P, F], f32, tag="uxy")
            V.tensor_scalar(uy[:], AY1[:], tcol(t, 1), None, op0=Alu.max)
            dy = tp.tile([P, F], f32, tag="dy")
            V.scalar_tensor_tensor(dy[:], AY2[:], tcol(t, 3), uy[:],
                                   op0=Alu.min, op1=Alu.subtract)
            dxr = tp.tile([P, F], f32, tag="dxr")
            S.activation(dxr[:], dx[:], Act.Relu)
            dyr = tp.tile([P, F], f32, tag="dyr")
            S.activation(dyr[:], dy[:], Act.Relu)
            inter = tp.tile([P, F], f32, tag="inter")
            V.tensor_mul(inter[:], dxr[:], dyr[:])
            union = tp.tile([P, F], f32, tag="union")
            V.scalar_tensor_tensor(union[:], AREAA[:], ABE[:, t : t + 1],
                                   inter[:], op0=Alu.add, op1=Alu.subtract)
            li = tp.tile([P, F], f32, tag="li")
            S.activation(li[:], inter[:], Act.Ln, bias=CTINY[:])
            lu = tp.tile([P, F], f32, tag="lu")
            S.activation(lu[:], union[:], Act.Ln)
            V.scalar_tensor_tensor(IOUtm[:, t], li[:],
                                   KEPS[:, t : t + 1], lu[:],
                                   op0=Alu.add, op1=Alu.subtract)
            V.max(RM8[:, ts(t, 8)], IOUtm[:, t])

        STAGE = int(os.environ.get("DETLOSS_STAGE", "9"))
        if STAGE < 2:
            V.tensor_copy(OUT[0:1, :], IOU[0:1, 0:2])
            continue

        if STAGE < 3:
            V.tensor_copy(OUT[0:1, :], mp[0:1, 0:2])
            continue
        # ---- per-anchor max over targets (contiguous inner reduce) ----
        V.tensor_reduce(mp[:], IOU[:].rearrange("p (f t) -> p f t", t=T),
                        axis=mybir.AxisListType.X, op=Alu.max)

        # ---- per-target global top-8 -> forced threshold ----
        # bounce through DRAM to transpose [p,(t,8)] -> [t,(p,8)]
        nc.gpsimd.dma_start(
            sc1_d[b].rearrange("(t p j) -> p t j", p=P, t=T),
            RM8[:].rearrange("p (t j) -> p t j", t=T))
        T8 = per.tile([T, P * 8], f32, tag="T8", bufs=1)
        nc.gpsimd.dma_start(T8[:], sc1_d[b].rearrange("(t x) -> t x", t=T))
        G8 = sm.tile([T, 8], f32)
        V.max(G8[:], T8[:])
        mx = sm.tile([T, 1], f32)
        V.tensor_max(mx[:], G8[:, 2:3], L03C[:])
        thr = sm.tile([T, 1], f32)
        V.tensor_tensor(thr[:], mx[:], G8[:, 0:1], op=Alu.min)
        nc.gpsimd.dma_start(sc2_d[b], thr[:])
        THRrow = sm.tile([1, T], f32)
        nc.gpsimd.dma_start(THRrow[:], sc2_d[b][None, :])
        THRrowV = sm.tile([1, T], f32, bufs=2)
        V.tensor_copy(THRrowV[:], THRrow[:])
        THR = per.tile([P, T], f32, tag="THR")
        thr_ps = ps.tile([P, T], f32, tag="bc_ps", bufs=1, name="thr_ps")
        PE.matmul(thr_ps[:], ONESROW[:], THRrowV[:], start=True, stop=True)
        V.tensor_copy(THR[:], thr_ps[:])

        if STAGE < 4:
            V.tensor_copy(OUT[0:1, :], THR[0:1, 0:2])
            continue
        # ---- pass 2: forced OR (reads IOU, leaves it intact) ----
        facc = per.tile([P, F], f32, tag="facc")
        for t in range(T):
            if t == 0:
                V.tensor_scalar(facc[:], IOUtm[:, 0], THR[:, 0:1], None,
                                op0=Alu.is_ge)
            else:
                V.scalar_tensor_tensor(facc[:], IOUtm[:, t],
                                       THR[:, t : t + 1], facc[:],
                                       op0=Alu.is_ge, op1=Alu.max)

        # ---- pass 3: eq into a small bf16 ring; PE transposes + matmuls ----
        mm_ps = [ps.tile([P, F], f32, tag="mm_ps", bufs=4, name=f"mm{q}")
                 for q in range(4)]
        M0 = per.tile([P, F], f32, tag="M0")
        M1 = per.tile([P, F], f32, tag="M1")
        M2 = per.tile([P, F], f32, tag="M2")
        M3 = per.tile([P, F], f32, tag="M3")
        IOU3 = IOU[:].rearrange("p (f t) -> p f t", t=T)
        for qq in range(8):
            EQB = per.tile([P, 16 * P], mybir.dt.bfloat16, tag="EQB", bufs=2)
            mpb = mp[:, 64 * qq : 64 * qq + 64].broadcast_to([P, 64, T])
            V.tensor_tensor(EQB[:].rearrange("p (f t) -> p f t", t=T),
                            IOU3[:, 64 * qq : 64 * qq + 64, :], mpb,
                            op=Alu.is_equal)
            for h in range(4):
                oh_ps = ps.tile([P, F], mybir.dt.bfloat16, tag="oh_ps",
                                bufs=2, name="oh_ps")
                for g in range(4):
                    PE.transpose(oh_ps[:, ts(g, P)],
                                 EQB[:, ts(h * 4 + g, P)], IDTB[:])
                OH = per.tile([P, F], f32, tag="OH", bufs=2)
                S.copy(OH[:], oh_ps[:])
                for g in range(4):
                    G = qq * 16 + h * 4 + g
                    q, Gl = G // 32, G % 32
                    PE.matmul(mm_ps[q][:, 16 * Gl : 16 * Gl + 16],
                              OH[:, ts(g, P)], TBDV[:], start=True, stop=True)
        for q in range(4):
            mmv = mm_ps[q][:].rearrange("p (G fp c) -> p c (G fp)", c=4, fp=4)
            S.copy(M0[:, ts(q, P)], mmv[:, 0])
            S.copy(M1[:, ts(q, P)], mmv[:, 1])
            S.copy(M2[:, ts(q, P)], mmv[:, 2])
            S.copy(M3[:, ts(q, P)], mmv[:, 3])

        # ---- epilogue (register style: R0..R5 scratch [P,F] tiles) ----
        R = [per.tile([P, F], f32, tag=f"R{i}", name=f"R{i}") for i in range(5)]
        R0, R1, R2, R3, R4 = [r[:] for r in R]
        R5 = R0
        COLS = per.tile([P, 8], f32, tag="COLS")
        V.memset(COLS[:], 0.0)

        pos = per.tile([P, F], f32, tag="pos")
        V.scalar_tensor_tensor(pos[:], mp[:], LN05, facc[:],
                               op0=Alu.is_ge, op1=Alu.max,
                               accum_out=COLS[:, 0:1])
        neg = per.tile([P, F], f32, tag="neg")
        V.tensor_scalar(R0, mp[:], LN04, None, op0=Alu.is_lt)
        V.tensor_sub(R0, R0, facc[:])
        V.tensor_scalar(neg[:], R0, 0.0, None, op0=Alu.max, op1=Alu.add,
                        accum_out=COLS[:, 1:2])

        # focal loss for all anchors: fl = (0.5*pos - 0.75) * (pt-1)^2 * ln(pt)
        fl = per.tile([P, F], f32, tag="fl")
        V.tensor_mul(R0, pos[:], CPt[:])
        V.scalar_tensor_tensor(R0, R0, 2.0, CPt[:],
                               op0=Alu.mult, op1=Alu.subtract)
        V.tensor_sub(R0, R0, pos[:])                 # pt - 1
        S.activation(R1, R0, Act.Ln, bias=1.0)       # ln(pt)
        S.activation(R2, R0, Act.Square)             # (1-pt)^2
        V.tensor_mul(R1, R1, R2)
        V.tensor_scalar(R0, pos[:], 0.5, 0.75, op0=Alu.mult, op1=Alu.subtract)
        V.tensor_mul(fl[:], R0, R1)
        V.scalar_tensor_tensor(R0, fl[:], 1.0, pos[:],
                               op0=Alu.mult, op1=Alu.mult,
                               accum_out=COLS[:, 2:3])
        ns = per.tile([P, F], f32, tag="ns")
        V.tensor_mul(ns[:], fl[:], neg[:])

        # giou + l1 on (bbox_pred, matched)
        V.tensor_max(R0, BX1[:], M0[:])
        V.tensor_tensor(R1, BX2[:], M2[:], op=Alu.min)
        V.tensor_sub(R1, R1, R0)
        V.tensor_scalar(R1, R1, 0.0, None, op0=Alu.max)   # dxc
        V.tensor_max(R0, BY1[:], M1[:])
        V.tensor_tensor(R2, BY2[:], M3[:], op=Alu.min)
        V.tensor_sub(R2, R2, R0)
        V.tensor_scalar(R2, R2, 0.0, None, op0=Alu.max)   # dyc
        V.tensor_mul(R1, R1, R2)                          # bint
        V.tensor_sub(R0, BX2[:], BX1[:])
        V.tensor_sub(R2, BY2[:], BY1[:])
        V.tensor_mul(R0, R0, R2)                          # areab
        V.tensor_sub(R2, M2[:], M0[:])
        V.tensor_sub(R3, M3[:], M1[:])
        V.tensor_mul(R2, R2, R3)                          # aream
        V.tensor_add(R0, R0, R2)
        V.tensor_sub(R0, R0, R1)                          # uni
        V.tensor_max(R2, BX2[:], M2[:])
        V.tensor_tensor(R3, BX1[:], M0[:], op=Alu.min)
        V.tensor_sub(R2, R2, R3)                          # dex
        V.tensor_max(R3, BY2[:], M3[:])
        V.tensor_tensor(R4, BY1[:], M1[:], op=Alu.min)
        V.tensor_sub(R3, R3, R4)                          # dey
        V.tensor_mul(R2, R2, R3)                          # enc
        V.tensor_scalar(R3, R0, EPS, None, op0=Alu.add)
        V.reciprocal(R3, R3)                              # 1/(uni+eps)
        V.tensor_mul(R1, R1, R3)                          # ioub
        V.tensor_scalar(R3, R2, EPS, None, op0=Alu.add)
        V.reciprocal(R3, R3)                              # 1/(enc+eps)
        V.tensor_sub(R2, R2, R0)                          # enc-uni
        V.tensor_mul(R2, R2, R3)
        V.tensor_sub(R2, R2, R1)                          # pb0 = (enc-uni)/(enc+e) - iou
        # L1 sum
        V.tensor_sub(R0, BX1[:], M0[:])
        S.activation(R0, R0, Act.Abs)
        V.tensor_sub(R1, BY1[:], M1[:])
        S.activation(R1, R1, Act.Abs)
        V.tensor_add(R0, R0, R1)
        V.tensor_sub(R1, BX2[:], M2[:])
        S.activation(R1, R1, Act.Abs)
        V.tensor_sub(R3, BY2[:], M3[:])
        S.activation(R3, R3, Act.Abs)
        V.tensor_add(R1, R1, R3)
        V.tensor_add(R0, R0, R1)                          # l1 sum
        V.scalar_tensor_tensor(R2, R0, 0.125, R2, op0=Alu.mult, op1=Alu.add)
        V.tensor_scalar(R2, R2, 1.0, None, op0=Alu.add)   # per_box
        V.scalar_tensor_tensor(R0, R2, 1.0, pos[:],
                               op0=Alu.mult, op1=Alu.mult,
                               accum_out=COLS[:, 3:4])

        # ---- reduce COLS across partitions ----
        pssum = ps.tile([1, 8], f32, tag="acc_ps", bufs=1, name="pssum")
        PE.matmul(pssum[:], ONES[:], COLS[:], start=True, stop=True)
        SU = sm.tile([1, 8], f32)
        V.tensor_copy(SU[:], pssum[:])
        npos = SU[:, 0:1]
        nneg = SU[:, 1:2]
        pfs = SU[:, 2:3]
        pbs = SU[:, 3:4]

        # k_neg = npos>0 ? min(nneg, 3*npos) : min(nneg, 100)   (on [1,1])
        k1 = sm.tile([1, 1], f32)
        V.tensor_scalar(k1[:], npos, 3.0, None, op0=Alu.mult)
        V.tensor_tensor(k1[:], k1[:], nneg, op=Alu.min)
        k0 = sm.tile([1, 1], f32)
        V.tensor_scalar(k0[:], nneg, 100.0, None, op0=Alu.min)
        zz = sm.tile([1, 1], f32)
        V.tensor_scalar(zz[:], npos, 0.5, None, op0=Alu.is_lt)
        kd = sm.tile([1, 1], f32)
        V.tensor_sub(kd[:], k0[:], k1[:])
        V.tensor_mul(kd[:], kd[:], zz[:])
        kk1 = sm.tile([1, 1], f32)
        V.tensor_add(kk1[:], k1[:], kd[:])
        kk_ps = ps.tile([P, 1], f32, tag="bc_ps", bufs=1, name="kk_ps")
        PE.matmul(kk_ps[:], ONESROW[:], kk1[:], start=True, stop=True)
        kk = per.tile([P, 1], f32, tag="kk")
        V.tensor_copy(kk[:], kk_ps[:])

        if STAGE < 6:
            V.tensor_copy(OUT[0:1, 0:1], kk[0:1, 0:1])
            continue
        # ---- binary search for k-th largest negative focal ----
        lo = per.tile([P, 1], f32, tag="lo")
        V.memset(lo[:], 0.0)
        step = 2.0
        for it in range(N_SEARCH):
            tau = sm.tile([P, 1], f32)
            V.tensor_scalar(tau[:], lo[:], step, None, op0=Alu.add)
            cntc = sm.tile([P, 1], f32)
            V.tensor_scalar(R5, ns[:], tau[:], None, op0=Alu.is_gt,
                            op1=Alu.add, accum_out=cntc[:])
            psc = ps.tile([1, 1], f32, tag="acc_ps", bufs=1, name="psc")
            PE.matmul(psc[:], ONES[:], cntc[:], start=True, stop=True)
            cb_ps = ps.tile([P, 1], f32, tag="bc_ps", bufs=1, name="cb_ps")
            crow = sm.tile([1, 1], f32)
            V.tensor_copy(crow[:], psc[:])
            PE.matmul(cb_ps[:], ONESROW[:], crow[:], start=True, stop=True)
            gg = sm.tile([P, 1], f32)
            V.tensor_tensor(gg[:], cb_ps[:], kk[:], op=Alu.is_ge)
            V.tensor_scalar(gg[:], gg[:], step, None, op0=Alu.mult)
            V.tensor_add(lo[:], lo[:], gg[:])
            step *= 0.5
        # final count + sum above lo
        cnt2 = sm.tile([P, 2], f32)
        V.tensor_scalar(R5, ns[:], lo[:], None, op0=Alu.is_gt,
                        op1=Alu.add, accum_out=cnt2[:, 0:1])
        V.scalar_tensor_tensor(R5, ns[:], lo[:], ns[:],
                               op0=Alu.is_gt, op1=Alu.mult,
                               accum_out=cnt2[:, 1:2])
        ps2 = ps.tile([1, 2], f32, tag="acc_ps", bufs=1, name="ps2")
        PE.matmul(ps2[:], ONES[:], cnt2[:], start=True, stop=True)
        C2 = sm.tile([1, 2], f32)
        V.tensor_copy(C2[:], ps2[:])
        nf = C2[:, 0:1]
        sf = C2[:, 1:2]

        # conf_loss = (pfs + sf + (k - nf)*lo) / max(npos + k, 1)
        kmn = sm.tile([1, 1], f32)
        V.tensor_sub(kmn[:], kk[0:1, :], nf)
        V.tensor_mul(kmn[:], kmn[:], lo[0:1, :])
        cnum = sm.tile([1, 1], f32)
        V.tensor_add(cnum[:], pfs, sf)
        V.tensor_add(cnum[:], cnum[:], kmn[:])
        den = sm.tile([1, 1], f32)
        V.tensor_add(den[:], npos, kk[0:1, :])
        V.tensor_scalar(den[:], den[:], 1.0, None, op0=Alu.max)
        rden = sm.tile([1, 1], f32)
        V.reciprocal(rden[:], den[:])
        confl = sm.tile([1, 1], f32)
        V.tensor_mul(confl[:], cnum[:], rden[:])

        # bbox_loss = npos>0 ? pbs / max(npos,1) : 0
        np1 = sm.tile([1, 1], f32)
        V.tensor_scalar(np1[:], npos, 1.0, None, op0=Alu.max)
        rnp = sm.tile([1, 1], f32)
        V.reciprocal(rnp[:], np1[:])
        bl0 = sm.tile([1, 1], f32)
        V.tensor_mul(bl0[:], pbs, rnp[:])
        zp = sm.tile([1, 1], f32)
        V.tensor_scalar(zp[:], npos, 0.5, None, op0=Alu.is_gt)
        bbl = sm.tile([1, 1], f32)
        V.tensor_mul(bbl[:], bl0[:], zp[:])

        V.tensor_add(OUT[0:1, 0:1], OUT[0:1, 0:1], confl[:])
        V.tensor_add(OUT[0:1, 1:2], OUT[0:1, 1:2], bbl[:])

    nc.gpsimd.dma_start(out_d, OUT[0:1, :])


def build():
    import concourse.bacc as bacc
    import concourse.mybir as mybir
    import concourse.tile as tile

    f32 = mybir.dt.float32
    nc = bacc.Bacc("TRN2", target_bir_lowering=False, debug=False)
    bp_d = nc.dram_tensor("bp", [BI, A, 4], f32, kind="ExternalInput")
    cp_d = nc.dram_tensor("cp", [BI, A], f32, kind="ExternalInput")
    an_d = nc.dram_tensor("an", [A, 4], f32, kind="ExternalInput")
    tb_d = nc.dram_tensor("tb", [BI, T, 4], f32, kind="ExternalInput")
    out_d = nc.dram_tensor("out", [2], f32, kind="ExternalOutput")
    sc1_d = nc.dram_tensor("scratch1", [BI, T * P * 8], f32)
    sc2_d = nc.dram_tensor("scratch2", [BI, T], f32)
    sc3_d = nc.dram_tensor("scratch3", [T], f32)
    sc4_d = nc.dram_tensor("scratch4", [P * 129], f32)
    with tile.TileContext(nc) as tc:
        with ExitStack() as ctx:
            _emit(nc, tc, ctx, bp_d.ap(), cp_d.ap(), an_d.ap(), tb_d.ap(),
                  out_d.ap(), sc1_d.ap(), sc2_d.ap(), sc3_d.ap(), sc4_d.ap())
    nc.compile()
    return nc


def kernel(bbox_pred, conf_pred, anchors, target_boxes):
    from concourse.bass_utils import run_bass_kernel_spmd

    bp = np.ascontiguousarray(np.asarray(bbox_pred, dtype=np.float32))
    cp = np.ascontiguousarray(np.asarray(conf_pred, dtype=np.float32))
    an = np.ascontiguousarray(np.asarray(anchors, dtype=np.float32))
    tb = np.ascontiguousarray(np.asarray(target_boxes, dtype=np.float32))

    nc = build()
    in_maps = []
    for k in range(NCORES):
        sl = slice(BI * k, BI * (k + 1))
        in_maps.append({"bp": bp[sl], "cp": cp[sl], "an": an, "tb": tb[sl]})
    trace = bool(int(os.environ.get("DETLOSS_TRACE", "0")))
    res = run_bass_kernel_spmd(nc, in_maps, list(range(NCORES)), trace=trace)
    partials = np.stack([res.results[k]["out"] for k in range(NCORES)])  # [8,2]
    conf = np.float32(partials[:, 0].sum() / 16.0)
    bbox = np.float32(partials[:, 1].sum() / 16.0)
    total = np.float32(conf + bbox)
    if trace:
        kernel.last_exec_time_ns = res.exec_time_ns
        kernel.last_trace = res.instructions_and_trace
    return (total, conf, bbox)



# revision 2
# speedup vs baseline: 1.0702x; 1.0702x over previous
"""DetectionLoss Trainium2 kernel, v2d.

Contract: kernel(**inputs) takes FULL inputs (bbox_pred [16,65536,4],
conf_pred [16,65536], anchors [65536,4], target_boxes [16,32,4]) and
returns the full output (total_loss, conf_loss, bbox_loss) as f32 scalars.

Sharding: data-parallel over batch. Core k processes images 2k, 2k+1 and
emits (sum conf_l, sum bbox_l); the host divides by 16 and sums.

v2d plan:
  - packed log-iou stored T-MAJOR; index-perturbation folded into the Ln
    scale f_t = exp((31-t)*EPSI).
  - interval algebra puts the per-target max on Scalar:
      dx = min(AX2-tx1, tw) - relu(AX1-tx1)
    with relu on S (bias = pre-negated target coord) and min as a
    dual-scalar 2x-mode tensor_scalar on V.
  - all bulk elementwise on V (GpSimd compute measured as a net loss:
    shared SBUF ports stall concurrent V 2x-mode ops ~1:1). GpSimd keeps
    only partition_all_reduce (280ns) for all cross-partition sums.
  - matched-box gather: eq blocks in (t,f) order, PE transposes + 16
    accumulating 128x128x256 fp16 matmuls per 64-f chunk against
    diagonal-mask rhs tiles (TBDG, built on S).
  - binary search 7 iters via partition_all_reduce.
  - cross-image software pipelining: img1's pass-1 groups are emitted
    between img0's tail stages (pass3/epilogue/binary search) so V chews
    intervals while img0's PE/S/DMA serial chains progress. IOU (bufs=1)
    hazard avoided by ordering img0's eq+pass2 before the interleave.
"""

import os
from contextlib import ExitStack

import numpy as np

P = 128          # SBUF partitions
F = 512          # anchors per partition row
T = 32           # targets per image
BI = 2           # images per core
A = P * F        # 65536 anchors
NCORES = 8

EPS = 1e-6
TINY = 1e-38
EPSI = float(2.0 ** -20)          # index-packing epsilon (log space)
DELTA = 2e-6                      # strictness margin for iou > 0.3
LN05 = float(np.log(0.5)) + 15.0 * EPSI
LN04 = float(np.log(0.4)) + 15.0 * EPSI
LN03D = float(np.log(0.3)) + DELTA
N_SEARCH = 6                      # binary-search iterations for kth value
F_T = [float(np.exp((31 - t) * EPSI)) for t in range(T)]


def _emit(nc, tc, ctx, bp_d, cp_d, an_d, tb_d, out_d, sc1_d, sc2_d, sc3_d,
          sc4_d, sc5_d, sc6_d):
    import concourse.bass as bass
    import concourse.bass_isa as bass_isa
    import concourse.mybir as mybir

    f32 = mybir.dt.float32
    f16 = mybir.dt.float16
    Alu = mybir.AluOpType
    Act = mybir.ActivationFunctionType
    Red = bass_isa.ReduceOp
    V = nc.vector
    S = nc.scalar
    G = nc.gpsimd
    PE = nc.tensor
    SY = nc.sync
    ts = bass.ts

    big = ctx.enter_context(tc.tile_pool(name="big", bufs=1))
    stage = ctx.enter_context(tc.tile_pool(name="stage", bufs=1))
    per = ctx.enter_context(tc.tile_pool(name="per", bufs=1))
    tp = ctx.enter_context(tc.tile_pool(name="tp", bufs=2))
    sm = ctx.enter_context(tc.tile_pool(name="sm", bufs=2))
    ps = ctx.enter_context(tc.tile_pool(name="ps", bufs=2, space="PSUM"))

    # ---- input loads first: keep the SP DMA queue free of const bounces
    AXYS = stage.tile([P, F * 4], f32, tag="BPS")
    anv = an_d.rearrange("(p f) c -> p (f c)", p=P)
    for _q in range(8):
        SY.dma_start(AXYS[:, ts(_q, F // 2)], anv[:, ts(_q, F // 2)])
    BPS0 = stage.tile([P, F * 4], f32, tag="BPS", bufs=1, name="BPS0")
    bpv0 = bp_d[0].rearrange("(p f) c -> p (f c)", p=P)
    for _q in range(8):
        SY.dma_start(BPS0[:, ts(_q, F // 2)], bpv0[:, ts(_q, F // 2)])
    CPt0 = per.tile([P, F], f32, tag="CPt", bufs=2, name="CPt0")
    SY.dma_start(CPt0[:], cp_d[0].rearrange("(p f) -> p f", p=P))
    TBrow0 = sm.tile([1, T * 4], f32, bufs=2, name="TBrow0")
    SY.dma_start(TBrow0[:], tb_d[0].rearrange("t c -> (t c)")[None, :])
    EV0 = sm.tile([2, 64], f32, bufs=2, name="EV0")
    SY.dma_start(EV0[:].rearrange("p (g c) -> p g c", c=4),
                 tb_d[0].rearrange("(g two) c -> two g c", two=2))
    preload = {0: (BPS0, CPt0, TBrow0, EV0)}

    AXv = AXYS[:].rearrange("p (f c) -> p c f", c=4)
    AX1 = big.tile([P, F], f16)
    AY1 = big.tile([P, F], f16)
    AX2 = big.tile([P, F], f16)
    AY2 = big.tile([P, F], f16)
    for _q in range(8):
        _sl = slice(64 * _q, 64 * (_q + 1))
        V.tensor_copy(AX1[:, _sl], AXv[:, 0, _sl])
        V.tensor_copy(AY1[:, _sl], AXv[:, 1, _sl])
        V.tensor_copy(AX2[:, _sl], AXv[:, 2, _sl])
        V.tensor_copy(AY2[:, _sl], AXv[:, 3, _sl])
    AREAA = big.tile([P, F], f16)
    aw0 = tp.tile([P, F], f16, tag="mx", bufs=3)
    ah0 = tp.tile([P, F], f16, tag="my", bufs=3)
    V.tensor_sub(aw0[:], AX2[:], AX1[:])
    V.tensor_sub(ah0[:], AY2[:], AY1[:])
    V.tensor_mul(AREAA[:], aw0[:], ah0[:])

    # per-target ln(0.3) + (31-t)*EPSI + DELTA on partitions 0..31
    L03row = sm.tile([1, T], f32)
    for t in range(T):
        V.memset(L03row[:, t : t + 1], LN03D + (31 - t) * EPSI)
    SY.dma_start(sc3_d, L03row[:])
    L03C = big.tile([T, 1], f32)
    SY.dma_start(L03C[:], sc3_d.rearrange("(t one) -> t one", one=1))

    ONES = big.tile([P, 1], f32)
    V.memset(ONES[:], 1.0)
    ONESROW = big.tile([1, P], f32)
    V.memset(ONESROW[:], 1.0)
    ONES32 = big.tile([T, P], f32)
    V.memset(ONES32[:], 1.0)
    # identity matrix via DRAM bounce: diag of [128,128] at stride 129
    ZED = stage.tile([P, P], f32, tag="ZED")
    V.memset(ZED[:], 0.0)
    SY.dma_start(sc4_d[: P * P].rearrange("(p f) -> p f", p=P), ZED[:])
    SY.dma_start(
        sc4_d[: P * 129].rearrange("(p f) -> p f", f=129)[:, 0:1], ONES[:])
    IDTV = stage.tile([P, P], f32, tag="IDTV", name="IDTV")
    SY.dma_start(IDTV[:], sc4_d[: P * P].rearrange("(p f) -> p f", p=P))
    IDTH = big.tile([P, P], f16)
    V.tensor_copy(IDTH[:], IDTV[:])

    # DIAG64 [P, 256] f32: D[p, c*64 + f] = (f == p % 64), via DRAM bounce.
    ZED2 = stage.tile([P, 256], f32, tag="ZED2")
    V.memset(ZED2[:], 0.0)
    SY.dma_start(sc5_d[: P * 256].rearrange("(p f) -> p f", p=P), ZED2[:])
    for half in range(2):
        for c in range(4):
            base = half * 64 * 256 + c * 64
            SY.dma_start(
                sc5_d[base : base + 64 * 257]
                .rearrange("(r x) -> r x", x=257)[:, 0:1],
                ONES[0:64, :])
    DIAG64 = big.tile([P, 256], f32)
    SY.dma_start(DIAG64[:], sc5_d[: P * 256].rearrange("(p f) -> p f", p=P))

    # lhsT for the 2-row broadcast matmul (TBALL build), via DRAM bounce
    ZROW = sm.tile([1, 256], f32)
    V.memset(ZROW[:, 0:64], 1.0)
    V.memset(ZROW[:, 64:192], 0.0)
    V.memset(ZROW[:, 192:256], 1.0)
    SY.dma_start(sc6_d, ZROW[:])
    LH2 = big.tile([2, P], f32)
    SY.dma_start(LH2[:], sc6_d.rearrange("(p f) -> p f", p=2))

    CTINY = big.tile([P, 1], f32)
    V.memset(CTINY[:], TINY)
    CEPS = big.tile([P, 1], f32)
    V.memset(CEPS[:], EPS)
    OUT = big.tile([1, 2], f32)
    V.memset(OUT[:], 0.0)

    IOU = big.tile([P, T * F], f16)     # packed log-iou (fp16), t-MAJOR

    def iou_t(t):
        return IOU[:, t * F : (t + 1) * F]

    st = [dict() for _ in range(BI)]

    # ================= per-image stages =================

    def prep(b):
        d = st[b]
        if b in preload:
            BPS, CPt, TBrow, EV = preload[b]
        else:
            BPS = stage.tile([P, F * 4], f32, tag="BPS", bufs=1, name="BPS")
            bpv = bp_d[b].rearrange("(p f) c -> p (f c)", p=P)
            for _q in range(4):
                SY.dma_start(BPS[:, ts(_q, F)], bpv[:, ts(_q, F)])
            CPt = per.tile([P, F], f32, tag="CPt", bufs=2, name="CPt")
            SY.dma_start(CPt[:], cp_d[b].rearrange("(p f) -> p f", p=P))
            TBrow = sm.tile([1, T * 4], f32, bufs=2, name="TBrow")
            SY.dma_start(TBrow[:], tb_d[b].rearrange("t c -> (t c)")[None, :])
            EV = sm.tile([2, 64], f32, bufs=2, name="EV")
            SY.dma_start(EV[:].rearrange("p (g c) -> p g c", c=4),
                         tb_d[b].rearrange("(g two) c -> two g c", two=2))
        d["BPS"] = BPS
        d["CPt"] = CPt
        d["EV"] = EV
        TBrowV = sm.tile([1, T * 4], f32, bufs=2, name="TBrowV")
        V.tensor_copy(TBrowV[:], TBrow[:])
        TBB = per.tile([P, T * 4], f32, tag="TBB", bufs=1, name="TBB")
        tbb_ps = ps.tile([P, T * 4], f32, tag="bc_ps", bufs=1, name="tbb_ps")
        PE.matmul(tbb_ps[:], ONESROW[:], TBrowV[:], start=True, stop=True)
        S.copy(TBB[:], tbb_ps[:])
        d["TBB"] = TBB
        NTB = per.tile([P, T * 4], f32, tag="NTB", bufs=1, name="NTB")
        V.tensor_scalar(NTB[:], TBB[:], -1.0, None, op0=Alu.mult)
        d["NTB"] = NTB
        TBv = TBB[:].rearrange("p (t c) -> p c t", c=4)
        TW = per.tile([P, T], f32, tag="TW", bufs=1, name="TW")
        TH = per.tile([P, T], f32, tag="TH", bufs=1, name="TH")
        ABE = per.tile([P, T], f32, tag="ABE", bufs=1, name="ABE")
        V.tensor_sub(TW[:], TBv[:, 2], TBv[:, 0])
        V.tensor_sub(TH[:], TBv[:, 3], TBv[:, 1])
        V.tensor_mul(ABE[:], TW[:], TH[:])
        V.tensor_scalar(ABE[:], ABE[:], EPS, None, op0=Alu.add)
        d["TW"], d["TH"], d["ABE"] = TW, TH, ABE

        # TBALL [P, 64]: row-block h=p//64 holds tb[2g+h, c] at col g*4+c
        EV = d["EV"]
        EVv = sm.tile([2, 64], f32, bufs=2, name="EVv")
        V.tensor_copy(EVv[:], EV[:])
        tball_ps = ps.tile([P, 64], f32, tag="bc_ps", bufs=1, name="tball_ps")
        PE.matmul(tball_ps[:], LH2[:], EVv[:], start=True, stop=True)
        TBALL = per.tile([P, 64], f32, tag="TBALL", bufs=2, name="TBALL")
        S.copy(TBALL[:], tball_ps[:])
        d["TBALL"] = TBALL

        d["mp"] = per.tile([P, F], f16, tag="mp", bufs=2, name="mp")
        d["RM8"] = per.tile([P, T * 8], f16, tag="RM8", bufs=1, name="RM8")
        d["groups"] = {}

    def load_bp(b):
        d = st[b]
        BPv = d["BPS"][:].rearrange("p (f c) -> p c f", c=4)
        BX1 = per.tile([P, F], f32, tag="BX1", bufs=1, name="BX1")
        BY1 = per.tile([P, F], f32, tag="BY1", bufs=1, name="BY1")
        BX2 = per.tile([P, F], f32, tag="BX2", bufs=1, name="BX2")
        BY2 = per.tile([P, F], f32, tag="BY2", bufs=1, name="BY2")
        S.copy(BX1[:], BPv[:, 0])
        S.copy(BY1[:], BPv[:, 1])
        S.copy(BX2[:], BPv[:, 2])
        S.copy(BY2[:], BPv[:, 3])
        d["BX1"], d["BY1"], d["BX2"], d["BY2"] = BX1, BY1, BX2, BY2

    def ncol(d, tt, c):
        return d["NTB"][:, 4 * tt + c : 4 * tt + c + 1]

    def pass1_group(b, g):
        d = st[b]
        t0, t1 = 2 * g, 2 * g + 1
        INTER2 = tp.tile([P, 2 * F], f16, tag="INTER2", bufs=3,
                         name=f"INTER2_{b}_{g}")
        UNION2 = tp.tile([P, 2 * F], f16, tag="UNION2", bufs=3,
                         name=f"UNION2_{b}_{g}")
        for k, t in enumerate((t0, t1)):
            # x: dx = min(AX2 - tx1, tw) - relu(AX1 - tx1)   (all fp16)
            rx = tp.tile([P, F], f16, tag="rx", bufs=3, name="rx")
            S.activation(rx[:], AX1[:], Act.Relu, bias=ncol(d, t, 0))
            mx = tp.tile([P, F], f16, tag="mx", bufs=3, name="mx")
            V.tensor_scalar(mx[:], AX2[:], ncol(d, t, 0),
                            d["TW"][:, t : t + 1], op0=Alu.add, op1=Alu.min)
            V.tensor_sub(mx[:], mx[:], rx[:])                    # dx
            # y: dy = min(AY2 - ty1, th) - relu(AY1 - ty1)
            ry = tp.tile([P, F], f16, tag="ry", bufs=3, name="ry")
            S.activation(ry[:], AY1[:], Act.Relu, bias=ncol(d, t, 1))
            my = tp.tile([P, F], f16, tag="my", bufs=3, name="my")
            V.tensor_scalar(my[:], AY2[:], ncol(d, t, 1),
                            d["TH"][:, t : t + 1], op0=Alu.add, op1=Alu.min)
            V.tensor_sub(my[:], my[:], ry[:])                    # dy
            V.tensor_scalar(my[:], my[:], 0.0, F_T[t],
                            op0=Alu.max, op1=Alu.mult)           # dyr (inpl)
            isl = INTER2[:, k * F : (k + 1) * F]
            V.scalar_tensor_tensor(isl, mx[:], 0.0, my[:],
                                   op0=Alu.max, op1=Alu.mult)    # inter
            V.tensor_sub(UNION2[:, k * F : (k + 1) * F], AREAA[:], isl)
        d["groups"][g] = (INTER2, UNION2)

    def packed_group(b, g):
        # emitted one group behind pass1_group so the S queue has the next
        # group's relus ahead of these Lns
        d = st[b]
        t0, t1 = 2 * g, 2 * g + 1
        INTER2, UNION2 = d["groups"].pop(g)
        ABE = d["ABE"]
        LI2 = tp.tile([P, 2 * F], f16, tag="LI2", bufs=3, name="LI2")
        LU2 = tp.tile([P, 2 * F], f16, tag="LU2", bufs=3, name="LU2")
        S.activation(LI2[:], INTER2[:], Act.Ln, bias=CTINY[:])
        S.activation(LU2[:, 0:F], UNION2[:, 0:F], Act.Ln,
                     bias=ABE[:, t0 : t0 + 1])
        S.activation(LU2[:, F : 2 * F], UNION2[:, F : 2 * F], Act.Ln,
                     bias=ABE[:, t1 : t1 + 1])
        V.tensor_tensor(IOU[:, t0 * F : (t1 + 1) * F], LI2[:],
                        LU2[:], op=Alu.subtract)
        V.max(d["RM8"][:, ts(t0, 8)], iou_t(t0))
        V.max(d["RM8"][:, ts(t1, 8)], iou_t(t1))
        mp = d["mp"]
        if g == 0:
            S.copy(mp[:], iou_t(0))
            V.tensor_max(mp[:], mp[:], iou_t(1))
        else:
            V.tensor_max(mp[:], mp[:], iou_t(t0))
            V.tensor_max(mp[:], mp[:], iou_t(t1))

    def thr_a(b):
        d = st[b]
        T8 = d["T8"]
        G8 = sm.tile([T, 8], f16, name="G8")
        V.max(G8[:], T8[:])
        G8F = sm.tile([T, 8], f32, name="G8F")
        S.copy(G8F[:], G8[:])
        mx8 = sm.tile([T, 1], f32, name="mx8")
        V.tensor_max(mx8[:], G8F[:, 2:3], L03C[:])
        thr = sm.tile([T, 1], f32, name="thr")
        V.tensor_tensor(thr[:], mx8[:], G8F[:, 0:1], op=Alu.min)
        # D32 = diag(thr): THR[i, j] = sum_p ones32[p, i] * D32[p, j] = thr[j]
        D32 = sm.tile([T, T], f32, name="D32")
        V.tensor_tensor(D32[:], IDTV[0:T, 0:T], thr[:].broadcast_to([T, T]),
                        op=Alu.mult)
        d["D32"] = D32

    def thr_b(b):
        d = st[b]
        THR = per.tile([P, T], f32, tag="THR", bufs=1, name="THR")
        thr_ps = ps.tile([P, T], f32, tag="bc_ps", bufs=1, name="thr_ps")
        PE.matmul(thr_ps[:], ONES32[:], d["D32"][:], start=True, stop=True)
        S.copy(THR[:], thr_ps[:])
        d["THR"] = THR

    def eq_pass(b):
        d = st[b]
        mp = d["mp"]
        IOU3 = IOU[:].rearrange("p (t f) -> p t f", t=T)
        eqbs = []
        for q in range(8):
            EQB = per.tile([P, T * 64], f16, tag="EQB", bufs=8, name="EQB")
            V.tensor_tensor(
                EQB[:].rearrange("p (t f) -> p t f", t=T),
                IOU3[:, :, q * 64 : (q + 1) * 64],
                mp[:, q * 64 : (q + 1) * 64]
                .rearrange("p (one f) -> p one f", one=1)
                .broadcast_to([P, T, 64]),
                op=Alu.is_equal)
            eqbs.append(EQB)
        d["eqbs"] = eqbs

    def pass2(b):
        d = st[b]
        THR = d["THR"]
        facc = per.tile([P, F], f32, tag="facc", bufs=1, name="facc")
        V.tensor_scalar(facc[:], iou_t(0), THR[:, 0:1], None, op0=Alu.is_ge)
        for t in range(1, T):
            V.scalar_tensor_tensor(facc[:], iou_t(t), THR[:, t : t + 1],
                                   facc[:], op0=Alu.is_ge, op1=Alu.max)
        d["facc"] = facc

    def tbdg_build(b):
        d = st[b]
        TBALL = d["TBALL"]
        TBDG = per.tile([P, 16 * 256], f16, tag="TBDG", bufs=1, name="TBDG")
        for g in range(16):
            for c in range(4):
                S.mul(TBDG[:, g * 256 + c * 64 : g * 256 + (c + 1) * 64],
                      DIAG64[:, c * 64 : (c + 1) * 64],
                      TBALL[:, 4 * g + c : 4 * g + c + 1])
        d["TBDG"] = TBDG

    def pass3(b):
        # PE + S only: transposes, gather matmuls, M copies
        d = st[b]
        TBDG = d["TBDG"]
        M0 = per.tile([P, F], f16, tag="M0", bufs=1, name="M0")
        M1 = per.tile([P, F], f16, tag="M1", bufs=1, name="M1")
        M2 = per.tile([P, F], f16, tag="M2", bufs=1, name="M2")
        M3 = per.tile([P, F], f16, tag="M3", bufs=1, name="M3")
        for q in range(8):
            EQB = d["eqbs"][q]
            mm_ps = ps.tile([P, 256], f32, tag="mm_ps", bufs=2, name="mm_ps")
            for h in range(4):
                oh_ps = ps.tile([P, 512], f16, tag="oh_ps", bufs=2,
                                name="oh_ps")
                for j in range(4):
                    PE.transpose(oh_ps[:, ts(j, P)],
                                 EQB[:, ts(4 * h + j, P)], IDTH[:])
                OH = per.tile([P, 512], f16, tag="OH", bufs=2, name="OH")
                S.copy(OH[:], oh_ps[:])
                for j in range(4):
                    g = 4 * h + j
                    PE.matmul(mm_ps[:], OH[:, ts(j, P)],
                              TBDG[:, g * 256 : (g + 1) * 256],
                              start=(g == 0), stop=(g == 15))
            S.copy(M0[:, ts(q, 64)], mm_ps[:, 0:64])
            S.copy(M1[:, ts(q, 64)], mm_ps[:, 64:128])
            S.copy(M2[:, ts(q, 64)], mm_ps[:, 128:192])
            S.copy(M3[:, ts(q, 64)], mm_ps[:, 192:256])
        d["M"] = (M0, M1, M2, M3)

    def epilogue1(b):
        d = st[b]
        mp, facc, CPt = d["mp"], d["facc"], d["CPt"]
        R = [per.tile([P, F], f32, tag=f"R{i}", bufs=1, name=f"R{i}")
             for i in range(4)]
        d["R"] = R
        R0, R1 = R[0][:], R[1][:]
        R2 = R[2][:]
        COLS = per.tile([P, 8], f32, tag="COLS", bufs=1, name="COLS")
        V.memset(COLS[:], 0.0)
        d["COLS"] = COLS

        pos = per.tile([P, F], f32, tag="pos", bufs=1, name="pos")
        V.scalar_tensor_tensor(pos[:], mp[:], LN05, facc[:],
                               op0=Alu.is_ge, op1=Alu.max,
                               accum_out=COLS[:, 0:1])
        neg = per.tile([P, F], f32, tag="neg", bufs=1, name="neg")
        V.scalar_tensor_tensor(R0, mp[:], LN04, facc[:],
                               op0=Alu.is_lt, op1=Alu.subtract)
        V.tensor_scalar(neg[:], R0, 0.0, None, op0=Alu.max, op1=Alu.add,
                        accum_out=COLS[:, 1:2])

        # focal: fl = (0.5*pos - 0.75) * (pt-1)^2 * ln(pt)
        fl = per.tile([P, F], f32, tag="fl", bufs=1, name="fl")
        V.tensor_mul(R0, pos[:], CPt[:])
        V.scalar_tensor_tensor(R0, R0, 2.0, CPt[:],
                               op0=Alu.mult, op1=Alu.subtract)
        V.tensor_sub(R0, R0, pos[:])                 # pt - 1
        S.activation(R1, R0, Act.Ln, bias=1.0)       # ln(pt)
        S.activation(R2, R0, Act.Square)             # (1-pt)^2
        V.tensor_mul(R1, R1, R2)
        V.tensor_scalar(R0, pos[:], 0.5, 0.75, op0=Alu.mult, op1=Alu.subtract)
        V.tensor_mul(fl[:], R0, R1)
        V.scalar_tensor_tensor(R0, fl[:], 1.0, pos[:],
                               op0=Alu.mult, op1=Alu.mult,
                               accum_out=COLS[:, 2:3])
        ns = per.tile([P, F], f32, tag="ns", bufs=1, name="ns")
        V.tensor_mul(ns[:], fl[:], neg[:])
        d["pos"], d["ns"], d["fl"] = pos, ns, fl

    def epilogue2(b):
        d = st[b]
        M0, M1, M2, M3 = d["M"]
        BX1, BY1, BX2, BY2 = d["BX1"][:], d["BY1"][:], d["BX2"][:], d["BY2"][:]
        R = d["R"]
        R0, R1, R2, R3 = [r[:] for r in R[:4]]
        R4 = d["fl"][:]
        V.tensor_max(R0, BX1, M0[:])
        V.tensor_tensor(R1, BX2, M2[:], op=Alu.min)
        V.tensor_sub(R1, R1, R0)
        V.tensor_scalar(R1, R1, 0.0, None, op0=Alu.max)   # dxc
        V.tensor_max(R0, BY1, M1[:])
        V.tensor_tensor(R2, BY2, M3[:], op=Alu.min)
        V.tensor_sub(R2, R2, R0)
        V.tensor_scalar(R2, R2, 0.0, None, op0=Alu.max)   # dyc
        V.tensor_mul(R1, R1, R2)                          # bint
        V.tensor_sub(R0, BX2, BX1)
        V.tensor_sub(R2, BY2, BY1)
        V.tensor_mul(R0, R0, R2)                          # areab
        V.tensor_sub(R2, M2[:], M0[:])
        V.tensor_sub(R3, M3[:], M1[:])
        V.tensor_mul(R2, R2, R3)                          # aream
        V.tensor_add(R0, R0, R2)
        V.tensor_sub(R0, R0, R1)                          # uni
        S.activation(R2, R0, Act.Ln, bias=CEPS[:])
        S.activation(R2, R2, Act.Exp, scale=-1.0)         # 1/(uni+eps)
        V.tensor_mul(R1, R1, R2)                          # ioub
        V.tensor_max(R2, BX2, M2[:])
        V.tensor_tensor(R3, BX1, M0[:], op=Alu.min)
        V.tensor_sub(R2, R2, R3)                          # dex
        V.tensor_max(R3, BY2, M3[:])
        V.tensor_tensor(R4, BY1, M1[:], op=Alu.min)
        V.tensor_sub(R3, R3, R4)                          # dey
        V.tensor_mul(R2, R2, R3)                          # enc
        S.activation(R3, R2, Act.Ln, bias=CEPS[:])
        S.activation(R3, R3, Act.Exp, scale=-1.0)         # 1/(enc+eps)
        V.tensor_sub(R2, R2, R0)                          # enc-uni
        V.tensor_mul(R2, R2, R3)
        V.tensor_sub(R2, R2, R1)                  # pb0 = (enc-uni)/(enc+e)-iou

    def epilogue3(b):
        d = st[b]
        M0, M1, M2, M3 = d["M"]
        BX1, BY1, BX2, BY2 = d["BX1"][:], d["BY1"][:], d["BX2"][:], d["BY2"][:]
        R = d["R"]
        R0, R1, R2, R3 = [r[:] for r in R[:4]]
        COLS, pos = d["COLS"], d["pos"]
        V.tensor_sub(R0, BX1, M0[:])
        S.activation(R0, R0, Act.Abs)
        V.tensor_sub(R1, BY1, M1[:])
        S.activation(R1, R1, Act.Abs)
        V.tensor_add(R0, R0, R1)
        V.tensor_sub(R1, BX2, M2[:])
        S.activation(R1, R1, Act.Abs)
        V.tensor_sub(R3, BY2, M3[:])
        S.activation(R3, R3, Act.Abs)
        V.tensor_add(R1, R1, R3)
        V.tensor_add(R0, R0, R1)                          # l1 sum
        V.scalar_tensor_tensor(R2, R0, 0.125, R2, op0=Alu.mult, op1=Alu.add)
        V.scalar_tensor_tensor(R3, R2, 1.0, pos[:],
                               op0=Alu.add, op1=Alu.mult,
                               accum_out=COLS[:, 3:4])

    def sums_kk(b):
        d = st[b]
        SUMS = sm.tile([P, 8], f32, name="SUMS")
        G.partition_all_reduce(SUMS[:], d["COLS"][:], P, Red.add)
        kk = per.tile([P, 1], f32, tag="kk", bufs=1, name="kk")
        npos = SUMS[:, 0:1]
        nneg = SUMS[:, 1:2]
        V.scalar_tensor_tensor(kk[:], npos, 3.0, nneg, op0=Alu.mult,
                               op1=Alu.min)
        k0 = sm.tile([P, 1], f32, name="k0")
        V.tensor_scalar(k0[:], nneg, 100.0, None, op0=Alu.min)
        zz = sm.tile([P, 1], f32, name="zz")
        V.tensor_scalar(zz[:], npos, 0.5, None, op0=Alu.is_lt)
        kd = sm.tile([P, 1], f32, name="kd")
        V.tensor_sub(kd[:], k0[:], kk[:])
        V.tensor_mul(kd[:], kd[:], zz[:])
        V.tensor_add(kk[:], kk[:], kd[:])
        d["kk"], d["SUMS"] = kk, SUMS
        lo = per.tile([P, 1], f32, tag="lo", bufs=1, name="lo")
        V.memset(lo[:], 0.0)
        d["lo"] = lo
        d["step"] = 2.0

    def bins_iter(b, it):
        d = st[b]
        ns, kk, lo = d["ns"], d["kk"], d["lo"]
        R5 = d["R"][0][:]
        step = d["step"]
        tau = sm.tile([P, 1], f32, name="tau")
        V.tensor_scalar(tau[:], lo[:], step, None, op0=Alu.add)
        cntc = sm.tile([P, 1], f32, name="cntc")
        V.tensor_scalar(R5, ns[:], tau[:], None, op0=Alu.is_gt,
                        op1=Alu.add, accum_out=cntc[:])
        cntb = sm.tile([P, 1], f32, name="cntb")
        G.partition_all_reduce(cntb[:], cntc[:], P, Red.add)
        gg = sm.tile([P, 1], f32, name="gg")
        V.tensor_tensor(gg[:], cntb[:], kk[:], op=Alu.is_ge)
        V.scalar_tensor_tensor(lo[:], gg[:], step, lo[:],
                               op0=Alu.mult, op1=Alu.add)
        d["step"] = step * 0.5

    def bins_final(b):
        d = st[b]
        ns, kk, lo, SUMS = d["ns"], d["kk"], d["lo"], d["SUMS"]
        R5 = d["R"][0][:]
        npos = SUMS[:, 0:1]
        pfs = SUMS[:, 2:3]
        pbs = SUMS[:, 3:4]
        cnt2 = sm.tile([P, 2], f32, name="cnt2")
        V.tensor_scalar(R5, ns[:], lo[:], None, op0=Alu.is_gt,
                        op1=Alu.add, accum_out=cnt2[:, 0:1])
        V.scalar_tensor_tensor(R5, ns[:], lo[:], ns[:],
                               op0=Alu.is_gt, op1=Alu.mult,
                               accum_out=cnt2[:, 1:2])
        C2 = sm.tile([P, 2], f32, name="C2")
        G.partition_all_reduce(C2[:], cnt2[:], P, Red.add)
        nf = C2[:, 0:1]
        sf = C2[:, 1:2]

        kmn = sm.tile([P, 1], f32, name="kmn")
        V.tensor_sub(kmn[:], kk[:], nf)
        V.tensor_mul(kmn[:], kmn[:], lo[:])
        cnum = sm.tile([P, 1], f32, name="cnum")
        V.tensor_add(cnum[:], pfs, sf)
        V.tensor_add(cnum[:], cnum[:], kmn[:])
        den = sm.tile([P, 1], f32, name="den")
        V.tensor_add(den[:], npos, kk[:])
        V.tensor_scalar(den[:], den[:], 1.0, None, op0=Alu.max)
        rden = sm.tile([P, 1], f32, name="rden")
        V.reciprocal(rden[:], den[:])
        confl = sm.tile([P, 1], f32, name="confl")
        V.tensor_mul(confl[:], cnum[:], rden[:])

        np1 = sm.tile([P, 1], f32, name="np1")
        V.tensor_scalar(np1[:], npos, 1.0, None, op0=Alu.max)
        rnp = sm.tile([P, 1], f32, name="rnp")
        V.reciprocal(rnp[:], np1[:])
        bl0 = sm.tile([P, 1], f32, name="bl0")
        V.tensor_mul(bl0[:], pbs, rnp[:])
        zp = sm.tile([P, 1], f32, name="zp")
        V.tensor_scalar(zp[:], npos, 0.5, None, op0=Alu.is_gt)
        bbl = sm.tile([P, 1], f32, name="bbl")
        V.tensor_mul(bbl[:], bl0[:], zp[:])

        V.tensor_add(OUT[0:1, 0:1], OUT[0:1, 0:1], confl[0:1, :])
        V.tensor_add(OUT[0:1, 1:2], OUT[0:1, 1:2], bbl[0:1, :])

    def pass1_all(b, tail_chunks=None):
        # pass1 groups; optional other-image tail chunks interleaved
        tail_chunks = list(tail_chunks or [])
        NG = T // 2
        ci = 0
        for g in range(NG):
            pass1_group(b, g)
            if g >= 1:
                packed_group(b, g - 1)
            if g >= 2 and ci < len(tail_chunks):
                tail_chunks[ci]()
                ci += 1
            if g == 8:
                # first half of RM8 (t=0..15) is final; start its bounce
                SY.dma_start(
                    sc1_d[b][: T * P * 4].rearrange(
                        "(t p j) -> p t j", p=P, t=T // 2),
                    st[b]["RM8"][:, : T * 4].rearrange(
                        "p (t j) -> p t j", t=T // 2))
            if g == 9:
                T8 = stage.tile([T, P * 8], f16, tag="BPS", bufs=1, name="T8")
                st[b]["T8"] = T8
                SY.dma_start(T8[0 : T // 2, :],
                             sc1_d[b][: T * P * 4].rearrange(
                                 "(t x) -> t x", t=T // 2))
        packed_group(b, NG - 1)
        SY.dma_start(
            sc1_d[b][T * P * 4 :].rearrange(
                "(t p j) -> p t j", p=P, t=T // 2),
            st[b]["RM8"][:, T * 4 :].rearrange(
                "p (t j) -> p t j", t=T // 2))
        SY.dma_start(st[b]["T8"][T // 2 : T, :],
                     sc1_d[b][T * P * 4 :].rearrange(
                         "(t x) -> t x", t=T // 2))
        while ci < len(tail_chunks):
            tail_chunks[ci]()
            ci += 1

    def tail_stages(b):
        chunks = [lambda: epilogue1(b), lambda: epilogue2(b),
                  lambda: epilogue3(b), lambda: sums_kk(b)]
        chunks += [(lambda it=it: bins_iter(b, it)) for it in range(N_SEARCH)]
        chunks += [lambda: bins_final(b)]
        return chunks

    # ================= schedule =================
    prep(0)
    load_bp(0)
    pass1_all(0)
    eq_pass(0)
    thr_a(0)
    tbdg_build(0)   # fills the sc2 bounce round trip
    thr_b(0)
    pass2(0)
    pass3(0)
    prep(1)
    # img1 pass1 interleaved with img0 tail; load_bp(1) overwrites the BX/BY
    # tiles img0's epilogue reads, so it must follow epilogue3(0)
    t0c = tail_stages(0)
    chunks = t0c[:3] + [lambda: load_bp(1)] + t0c[3:]
    during, after = chunks[:-3], chunks[-3:]
    pass1_all(1, tail_chunks=during)
    eq_pass(1)
    thr_a(1)
    after[0]()
    tbdg_build(1)
    after[1]()
    thr_b(1)
    after[2]()
    pass2(1)
    pass3(1)
    for c in tail_stages(1):
        c()

    SY.dma_start(out_d, OUT[0:1, :])


def build():
    import concourse.bacc as bacc
    import concourse.mybir as mybir
    import concourse.tile as tile

    f32 = mybir.dt.float32
    nc = bacc.Bacc("TRN2", target_bir_lowering=False, debug=False)
    bp_d = nc.dram_tensor("bp", [BI, A, 4], f32, kind="ExternalInput")
    cp_d = nc.dram_tensor("cp", [BI, A], f32, kind="ExternalInput")
    an_d = nc.dram_tensor("an", [A, 4], f32, kind="ExternalInput")
    tb_d = nc.dram_tensor("tb", [BI, T, 4], f32, kind="ExternalInput")
    out_d = nc.dram_tensor("out", [2], f32, kind="ExternalOutput")
    sc1_d = nc.dram_tensor("scratch1", [BI, T * P * 8], mybir.dt.float16)
    sc2_d = nc.dram_tensor("scratch2", [BI, T], f32)
    sc3_d = nc.dram_tensor("scratch3", [T], f32)
    sc4_d = nc.dram_tensor("scratch4", [P * 129], f32)
    sc5_d = nc.dram_tensor("scratch5", [P * 256 + 256], f32)
    sc6_d = nc.dram_tensor("scratch6", [256], f32)
    with tile.TileContext(nc) as tc:
        with ExitStack() as ctx:
            _emit(nc, tc, ctx, bp_d.ap(), cp_d.ap(), an_d.ap(), tb_d.ap(),
                  out_d.ap(), sc1_d.ap(), sc2_d.ap(), sc3_d.ap(), sc4_d.ap(),
                  sc5_d.ap(), sc6_d.ap())
    nc.compile()
    return nc


def kernel(bbox_pred, conf_pred, anchors, target_boxes):
    from concourse.bass_utils import run_bass_kernel_spmd

    bp = np.ascontiguousarray(np.asarray(bbox_pred, dtype=np.float32))
    cp = np.ascontiguousarray(np.asarray(conf_pred, dtype=np.float32))
    an = np.ascontiguousarray(np.asarray(anchors, dtype=np.float32))
    tb = np.ascontiguousarray(np.asarray(target_boxes, dtype=np.float32))

    nc = build()
    in_maps = []
    for k in range(NCORES):
        sl = slice(BI * k, BI * (k + 1))
        in_maps.append({"bp": bp[sl], "cp": cp[sl], "an": an, "tb": tb[sl]})
    trace = bool(int(os.environ.get("DETLOSS_TRACE", "0")))
    res = run_bass_kernel_spmd(nc, in_maps, list(range(NCORES)), trace=trace)
    partials = np.stack([res.results[k]["out"] for k in range(NCORES)])  # [8,2]
    conf = np.float32(partials[:, 0].sum() / 16.0)
    bbox = np.float32(partials[:, 1].sum() / 16.0)
    total = np.float32(conf + bbox)
    if trace:
        kernel.last_exec_time_ns = res.exec_time_ns
        kernel.last_trace = res.instructions_and_trace
    return (total, conf, bbox)


# revision 3
# speedup vs baseline: 1.0793x; 1.0086x over previous
"""DetectionLoss Trainium2 kernel, v2d.

Contract: kernel(**inputs) takes FULL inputs (bbox_pred [16,65536,4],
conf_pred [16,65536], anchors [65536,4], target_boxes [16,32,4]) and
returns the full output (total_loss, conf_loss, bbox_loss) as f32 scalars.

Sharding: data-parallel over batch. Core k processes images 2k, 2k+1 and
emits (sum conf_l, sum bbox_l); the host divides by 16 and sums.

v2d plan:
  - packed log-iou stored T-MAJOR; index-perturbation folded into the Ln
    scale f_t = exp((31-t)*EPSI).
  - interval algebra puts the per-target max on Scalar:
      dx = min(AX2-tx1, tw) - relu(AX1-tx1)
    with relu on S (bias = pre-negated target coord) and min as a
    dual-scalar 2x-mode tensor_scalar on V.
  - all bulk elementwise on V (GpSimd compute measured as a net loss:
    shared SBUF ports stall concurrent V 2x-mode ops ~1:1). GpSimd keeps
    only partition_all_reduce (280ns) for all cross-partition sums.
  - matched-box gather: eq blocks in (t,f) order, PE transposes + 16
    accumulating 128x128x256 fp16 matmuls per 64-f chunk against
    diagonal-mask rhs tiles (TBDG, built on S).
  - binary search 7 iters via partition_all_reduce.
  - cross-image software pipelining: img1's pass-1 groups are emitted
    between img0's tail stages (pass3/epilogue/binary search) so V chews
    intervals while img0's PE/S/DMA serial chains progress. IOU (bufs=1)
    hazard avoided by ordering img0's eq+pass2 before the interleave.
"""

import os
from contextlib import ExitStack

import numpy as np

P = 128          # SBUF partitions
F = 512          # anchors per partition row
T = 32           # targets per image
BI = 2           # images per core
A = P * F        # 65536 anchors
NCORES = 8

EPS = 1e-6
TINY = 1e-38
EPSI = float(2.0 ** -20)          # index-packing epsilon (log space)
DELTA = 2e-6                      # strictness margin for iou > 0.3
LN05 = float(np.log(0.5)) + 15.0 * EPSI
LN04 = float(np.log(0.4)) + 15.0 * EPSI
LN03D = float(np.log(0.3)) + DELTA
N_SEARCH = 5                      # binary-search iterations for kth value
F_T = [float(np.exp((31 - t) * EPSI)) for t in range(T)]


def _emit(nc, tc, ctx, bp_d, cp_d, an_d, tb_d, out_d, sc1_d, sc2_d, sc3_d,
          sc4_d, sc5_d, sc6_d):
    import concourse.bass as bass
    import concourse.bass_isa as bass_isa
    import concourse.mybir as mybir

    f32 = mybir.dt.float32
    f16 = mybir.dt.float16
    Alu = mybir.AluOpType
    Act = mybir.ActivationFunctionType
    Red = bass_isa.ReduceOp
    V = nc.vector
    S = nc.scalar
    G = nc.gpsimd
    PE = nc.tensor
    SY = nc.sync
    ts = bass.ts

    big = ctx.enter_context(tc.tile_pool(name="big", bufs=1))
    stage = ctx.enter_context(tc.tile_pool(name="stage", bufs=1))
    per = ctx.enter_context(tc.tile_pool(name="per", bufs=1))
    tp = ctx.enter_context(tc.tile_pool(name="tp", bufs=2))
    sm = ctx.enter_context(tc.tile_pool(name="sm", bufs=2))
    ps = ctx.enter_context(tc.tile_pool(name="ps", bufs=2, space="PSUM"))

    # ---- input loads first: smalls, then big chunked loads
    TBrow0 = sm.tile([1, T * 4], f32, bufs=2, name="TBrow0")
    SY.dma_start(TBrow0[:], tb_d[0].rearrange("t c -> (t c)")[None, :])
    EV0 = sm.tile([2, 64], f32, bufs=2, name="EV0")
    SY.dma_start(EV0[:].rearrange("p (g c) -> p g c", c=4),
                 tb_d[0].rearrange("(g two) c -> two g c", two=2))
    CPt0 = per.tile([P, F], f32, tag="CPt", bufs=2, name="CPt0")
    SY.dma_start(CPt0[:], cp_d[0].rearrange("(p f) -> p f", p=P))
    AXYS = stage.tile([P, F * 4], f32, tag="BPS")
    anv = an_d.rearrange("(p f) c -> p (f c)", p=P)
    for _q in range(8):
        SY.dma_start(AXYS[:, ts(_q, F // 2)], anv[:, ts(_q, F // 2)])
    BPS0 = stage.tile([P, F * 4], f32, tag="BPS", bufs=1, name="BPS0")
    bpv0 = bp_d[0].rearrange("(p f) c -> p (f c)", p=P)
    for _q in range(8):
        SY.dma_start(BPS0[:, ts(_q, F // 2)], bpv0[:, ts(_q, F // 2)])
    preload = {0: (BPS0, CPt0, TBrow0, EV0)}

    AXv = AXYS[:].rearrange("p (f c) -> p c f", c=4)
    AX1 = big.tile([P, F], f16)
    AY1 = big.tile([P, F], f16)
    AX2 = big.tile([P, F], f16)
    AY2 = big.tile([P, F], f16)

    def anchor_casts():
        for _q in range(8):
            _sl = slice(64 * _q, 64 * (_q + 1))
            V.tensor_copy(AX1[:, _sl], AXv[:, 0, _sl])
            V.tensor_copy(AY1[:, _sl], AXv[:, 1, _sl])
            V.tensor_copy(AX2[:, _sl], AXv[:, 2, _sl])
            V.tensor_copy(AY2[:, _sl], AXv[:, 3, _sl])
    AREAA = big.tile([P, F], f16)

    def areaa_build():
        aw0 = tp.tile([P, F], f16, tag="mx", bufs=3)
        ah0 = tp.tile([P, F], f16, tag="my", bufs=3)
        V.tensor_sub(aw0[:], AX2[:], AX1[:])
        V.tensor_sub(ah0[:], AY2[:], AY1[:])
        V.tensor_mul(AREAA[:], aw0[:], ah0[:])

    # per-target ln(0.3) + (31-t)*EPSI + DELTA on partitions 0..31
    L03row = sm.tile([1, T], f32)
    for t in range(T):
        V.memset(L03row[:, t : t + 1], LN03D + (31 - t) * EPSI)
    SY.dma_start(sc3_d, L03row[:])
    L03C = big.tile([T, 1], f32)
    SY.dma_start(L03C[:], sc3_d.rearrange("(t one) -> t one", one=1))

    ONES = big.tile([P, 1], f32)
    V.memset(ONES[:], 1.0)
    ONESROW = big.tile([1, P], f32)
    V.memset(ONESROW[:], 1.0)
    ONES32 = big.tile([T, P], f32)
    V.memset(ONES32[:], 1.0)
    # identity matrix via DRAM bounce: diag of [128,128] at stride 129
    ZED = stage.tile([P, P], f32, tag="ZED")
    V.memset(ZED[:], 0.0)
    SY.dma_start(sc4_d[: P * P].rearrange("(p f) -> p f", p=P), ZED[:])
    SY.dma_start(
        sc4_d[: P * 129].rearrange("(p f) -> p f", f=129)[:, 0:1], ONES[:])
    IDTV = stage.tile([P, P], f32, tag="IDTV", name="IDTV")
    SY.dma_start(IDTV[:], sc4_d[: P * P].rearrange("(p f) -> p f", p=P))
    IDTH = big.tile([P, P], f16)

    def late_setup():
        # IDTV bounce has long landed by now; no V head-block
        V.tensor_copy(IDTH[:], IDTV[:])

    # DIAG64 [P, 256] f32: D[p, c*64 + f] = (f == p % 64), via DRAM bounce.
    ZED2 = stage.tile([P, 256], f32, tag="ZED2")
    V.memset(ZED2[:], 0.0)
    SY.dma_start(sc5_d[: P * 256].rearrange("(p f) -> p f", p=P), ZED2[:])
    for half in range(2):
        for c in range(4):
            base = half * 64 * 256 + c * 64
            SY.dma_start(
                sc5_d[base : base + 64 * 257]
                .rearrange("(r x) -> r x", x=257)[:, 0:1],
                ONES[0:64, :])
    DIAG64 = big.tile([P, 256], f32)
    SY.dma_start(DIAG64[:], sc5_d[: P * 256].rearrange("(p f) -> p f", p=P))

    # lhsT for the 2-row broadcast matmul (TBALL build), via DRAM bounce
    ZROW = sm.tile([1, 256], f32)
    V.memset(ZROW[:, 0:64], 1.0)
    V.memset(ZROW[:, 64:192], 0.0)
    V.memset(ZROW[:, 192:256], 1.0)
    SY.dma_start(sc6_d, ZROW[:])
    LH2 = big.tile([2, P], f32)
    SY.dma_start(LH2[:], sc6_d.rearrange("(p f) -> p f", p=2))

    CTINY = big.tile([P, 1], f32)
    V.memset(CTINY[:], TINY)
    CEPS = big.tile([P, 1], f32)
    V.memset(CEPS[:], EPS)
    OUT = big.tile([1, 2], f32)
    V.memset(OUT[:], 0.0)

    IOU = big.tile([P, T * F], f16)     # packed log-iou (fp16), t-MAJOR

    def iou_t(t):
        return IOU[:, t * F : (t + 1) * F]

    st = [dict() for _ in range(BI)]

    # ================= per-image stages =================

    def prep(b):
        d = st[b]
        if b in preload:
            BPS, CPt, TBrow, EV = preload[b]
        else:
            BPS = stage.tile([P, F * 4], f32, tag="BPS", bufs=1, name="BPS")
            bpv = bp_d[b].rearrange("(p f) c -> p (f c)", p=P)
            for _q in range(4):
                SY.dma_start(BPS[:, ts(_q, F)], bpv[:, ts(_q, F)])
            CPt = per.tile([P, F], f32, tag="CPt", bufs=2, name="CPt")
            SY.dma_start(CPt[:], cp_d[b].rearrange("(p f) -> p f", p=P))
            TBrow = sm.tile([1, T * 4], f32, bufs=2, name="TBrow")
            SY.dma_start(TBrow[:], tb_d[b].rearrange("t c -> (t c)")[None, :])
            EV = sm.tile([2, 64], f32, bufs=2, name="EV")
            SY.dma_start(EV[:].rearrange("p (g c) -> p g c", c=4),
                         tb_d[b].rearrange("(g two) c -> two g c", two=2))
        d["BPS"] = BPS
        d["CPt"] = CPt
        d["EV"] = EV
        TBrowV = sm.tile([1, T * 4], f32, bufs=2, name="TBrowV")
        V.tensor_copy(TBrowV[:], TBrow[:])
        TBB = per.tile([P, T * 4], f32, tag="TBB", bufs=1, name="TBB")
        tbb_ps = ps.tile([P, T * 4], f32, tag="bc_ps", bufs=1, name="tbb_ps")
        PE.matmul(tbb_ps[:], ONESROW[:], TBrowV[:], start=True, stop=True)
        S.copy(TBB[:], tbb_ps[:])
        d["TBB"] = TBB
        NTB = per.tile([P, T * 4], f32, tag="NTB", bufs=1, name="NTB")
        V.tensor_scalar(NTB[:], TBB[:], -1.0, None, op0=Alu.mult)
        d["NTB"] = NTB
        TBv = TBB[:].rearrange("p (t c) -> p c t", c=4)
        TW = per.tile([P, T], f32, tag="TW", bufs=1, name="TW")
        TH = per.tile([P, T], f32, tag="TH", bufs=1, name="TH")
        ABE = per.tile([P, T], f32, tag="ABE", bufs=1, name="ABE")
        V.tensor_sub(TW[:], TBv[:, 2], TBv[:, 0])
        V.tensor_sub(TH[:], TBv[:, 3], TBv[:, 1])
        V.tensor_mul(ABE[:], TW[:], TH[:])
        V.tensor_scalar(ABE[:], ABE[:], EPS, None, op0=Alu.add)
        d["TW"], d["TH"], d["ABE"] = TW, TH, ABE

        d["mp"] = per.tile([P, F], f16, tag="mp", bufs=2, name="mp")
        d["RM8"] = per.tile([P, T * 8], f16, tag="RM8", bufs=1, name="RM8")
        d["groups"] = {}

    def load_bp(b):
        d = st[b]
        BPv = d["BPS"][:].rearrange("p (f c) -> p c f", c=4)
        BX1 = per.tile([P, F], f16, tag="BX1", bufs=1, name="BX1")
        BY1 = per.tile([P, F], f16, tag="BY1", bufs=1, name="BY1")
        BX2 = per.tile([P, F], f16, tag="BX2", bufs=1, name="BX2")
        BY2 = per.tile([P, F], f16, tag="BY2", bufs=1, name="BY2")
        S.copy(BX1[:], BPv[:, 0])
        S.copy(BY1[:], BPv[:, 1])
        S.copy(BX2[:], BPv[:, 2])
        S.copy(BY2[:], BPv[:, 3])
        d["BX1"], d["BY1"], d["BX2"], d["BY2"] = BX1, BY1, BX2, BY2

    def ncol(d, tt, c):
        return d["NTB"][:, 4 * tt + c : 4 * tt + c + 1]

    def pass1_group(b, g):
        d = st[b]
        t0, t1 = 2 * g, 2 * g + 1
        INTER2 = tp.tile([P, 2 * F], f16, tag="INTER2", bufs=3,
                         name=f"INTER2_{b}_{g}")
        UNION2 = tp.tile([P, 2 * F], f16, tag="UNION2", bufs=3,
                         name=f"UNION2_{b}_{g}")
        for k, t in enumerate((t0, t1)):
            # x: dx = min(AX2 - tx1, tw) - relu(AX1 - tx1)   (all fp16)
            rx = tp.tile([P, F], f16, tag="rx", bufs=3, name="rx")
            S.activation(rx[:], AX1[:], Act.Relu, bias=ncol(d, t, 0))
            mx = tp.tile([P, F], f16, tag="mx", bufs=3, name="mx")
            V.tensor_scalar(mx[:], AX2[:], ncol(d, t, 0),
                            d["TW"][:, t : t + 1], op0=Alu.add, op1=Alu.min)
            V.tensor_sub(mx[:], mx[:], rx[:])                    # dx
            # y: dy = min(AY2 - ty1, th) - relu(AY1 - ty1)
            ry = tp.tile([P, F], f16, tag="ry", bufs=3, name="ry")
            S.activation(ry[:], AY1[:], Act.Relu, bias=ncol(d, t, 1))
            my = tp.tile([P, F], f16, tag="my", bufs=3, name="my")
            V.tensor_scalar(my[:], AY2[:], ncol(d, t, 1),
                            d["TH"][:, t : t + 1], op0=Alu.add, op1=Alu.min)
            V.tensor_sub(my[:], my[:], ry[:])                    # dy
            V.tensor_scalar(my[:], my[:], 0.0, F_T[t],
                            op0=Alu.max, op1=Alu.mult)           # dyr (inpl)
            isl = INTER2[:, k * F : (k + 1) * F]
            V.scalar_tensor_tensor(isl, mx[:], 0.0, my[:],
                                   op0=Alu.max, op1=Alu.mult)    # inter
            V.tensor_sub(UNION2[:, k * F : (k + 1) * F], AREAA[:], isl)
        d["groups"][g] = (INTER2, UNION2)

    def packed_group(b, g):
        # emitted one group behind pass1_group so the S queue has the next
        # group's relus ahead of these Lns
        d = st[b]
        t0, t1 = 2 * g, 2 * g + 1
        INTER2, UNION2 = d["groups"].pop(g)
        ABE = d["ABE"]
        LI2 = tp.tile([P, 2 * F], f16, tag="LI2", bufs=3, name="LI2")
        LU2 = tp.tile([P, 2 * F], f16, tag="LU2", bufs=3, name="LU2")
        S.activation(LI2[:], INTER2[:], Act.Ln, bias=CTINY[:])
        S.activation(LU2[:, 0:F], UNION2[:, 0:F], Act.Ln,
                     bias=ABE[:, t0 : t0 + 1])
        S.activation(LU2[:, F : 2 * F], UNION2[:, F : 2 * F], Act.Ln,
                     bias=ABE[:, t1 : t1 + 1])
        V.tensor_tensor(IOU[:, t0 * F : (t1 + 1) * F], LI2[:],
                        LU2[:], op=Alu.subtract)
        V.max(d["RM8"][:, ts(t0, 8)], iou_t(t0))
        V.max(d["RM8"][:, ts(t1, 8)], iou_t(t1))
        mp = d["mp"]
        if g == 0:
            S.copy(mp[:], iou_t(0))
            V.tensor_max(mp[:], mp[:], iou_t(1))
        else:
            V.tensor_max(mp[:], mp[:], iou_t(t0))
            V.tensor_max(mp[:], mp[:], iou_t(t1))

    def thr_a(b):
        d = st[b]
        T8 = d["T8"]
        G8 = sm.tile([T, 8], f16, name="G8")
        V.max(G8[:], T8[:])
        G8F = sm.tile([T, 8], f32, name="G8F")
        S.copy(G8F[:], G8[:])
        mx8 = sm.tile([T, 1], f32, name="mx8")
        V.tensor_max(mx8[:], G8F[:, 2:3], L03C[:])
        thr = sm.tile([T, 1], f32, name="thr")
        V.tensor_tensor(thr[:], mx8[:], G8F[:, 0:1], op=Alu.min)
        # D32 = diag(thr): THR[i, j] = sum_p ones32[p, i] * D32[p, j] = thr[j]
        D32 = sm.tile([T, T], f32, name="D32")
        V.tensor_tensor(D32[:], IDTV[0:T, 0:T], thr[:].broadcast_to([T, T]),
                        op=Alu.mult)
        d["D32"] = D32

    def thr_b(b):
        d = st[b]
        THR = per.tile([P, T], f32, tag="THR", bufs=1, name="THR")
        thr_ps = ps.tile([P, T], f32, tag="bc_ps", bufs=1, name="thr_ps")
        PE.matmul(thr_ps[:], ONES32[:], d["D32"][:], start=True, stop=True)
        S.copy(THR[:], thr_ps[:])
        d["THR"] = THR

    def eq_pass(b):
        d = st[b]
        mp = d["mp"]
        IOU3 = IOU[:].rearrange("p (t f) -> p t f", t=T)
        eqbs = []
        for q in range(8):
            EQB = per.tile([P, T * 64], f16, tag="EQB", bufs=8, name="EQB")
            V.tensor_tensor(
                EQB[:].rearrange("p (t f) -> p t f", t=T),
                IOU3[:, :, q * 64 : (q + 1) * 64],
                mp[:, q * 64 : (q + 1) * 64]
                .rearrange("p (one f) -> p one f", one=1)
                .broadcast_to([P, T, 64]),
                op=Alu.is_equal)
            eqbs.append(EQB)
        d["eqbs"] = eqbs

    def pass2(b):
        d = st[b]
        THR = d["THR"]
        facc = per.tile([P, F], f32, tag="facc", bufs=1, name="facc")
        V.tensor_scalar(facc[:], iou_t(0), THR[:, 0:1], None, op0=Alu.is_ge)
        for t in range(1, T):
            V.scalar_tensor_tensor(facc[:], iou_t(t), THR[:, t : t + 1],
                                   facc[:], op0=Alu.is_ge, op1=Alu.max)
        d["facc"] = facc

    def tbdg_build(b):
        d = st[b]
        # TBALL [P, 64]: row-block h=p//64 holds tb[2g+h, c] at col g*4+c
        EV = d["EV"]
        EVv = sm.tile([2, 64], f32, bufs=2, name="EVv")
        V.tensor_copy(EVv[:], EV[:])
        tball_ps = ps.tile([P, 64], f32, tag="bc_ps", bufs=1, name="tball_ps")
        PE.matmul(tball_ps[:], LH2[:], EVv[:], start=True, stop=True)
        TBALL = per.tile([P, 64], f32, tag="TBALL", bufs=2, name="TBALL")
        S.copy(TBALL[:], tball_ps[:])
        TBDG = per.tile([P, 16 * 256], f16, tag="TBDG", bufs=1, name="TBDG")
        for g in range(16):
            for c in range(4):
                S.mul(TBDG[:, g * 256 + c * 64 : g * 256 + (c + 1) * 64],
                      DIAG64[:, c * 64 : (c + 1) * 64],
                      TBALL[:, 4 * g + c : 4 * g + c + 1])
        d["TBDG"] = TBDG

    def pass3(b):
        # PE + S only: transposes, gather matmuls, M copies
        d = st[b]
        TBDG = d["TBDG"]
        M0 = per.tile([P, F], f16, tag="M0", bufs=1, name="M0")
        M1 = per.tile([P, F], f16, tag="M1", bufs=1, name="M1")
        M2 = per.tile([P, F], f16, tag="M2", bufs=1, name="M2")
        M3 = per.tile([P, F], f16, tag="M3", bufs=1, name="M3")
        for q in range(8):
            EQB = d["eqbs"][q]
            mm_ps = ps.tile([P, 256], f32, tag="mm_ps", bufs=2, name="mm_ps")
            for h in range(4):
                oh_ps = ps.tile([P, 512], f16, tag="oh_ps", bufs=2,
                                name="oh_ps")
                for j in range(4):
                    PE.transpose(oh_ps[:, ts(j, P)],
                                 EQB[:, ts(4 * h + j, P)], IDTH[:])
                OH = per.tile([P, 512], f16, tag="OH", bufs=2, name="OH")
                S.copy(OH[:], oh_ps[:])
                for j in range(4):
                    g = 4 * h + j
                    PE.matmul(mm_ps[:], OH[:, ts(j, P)],
                              TBDG[:, g * 256 : (g + 1) * 256],
                              start=(g == 0), stop=(g == 15))
            S.copy(M0[:, ts(q, 64)], mm_ps[:, 0:64])
            S.copy(M1[:, ts(q, 64)], mm_ps[:, 64:128])
            S.copy(M2[:, ts(q, 64)], mm_ps[:, 128:192])
            S.copy(M3[:, ts(q, 64)], mm_ps[:, 192:256])
        d["M"] = (M0, M1, M2, M3)

    def epilogue1(b):
        d = st[b]
        mp, facc, CPt = d["mp"], d["facc"], d["CPt"]
        R = [per.tile([P, F], f32, tag=f"R{i}", bufs=1, name=f"R{i}")
             for i in range(4)]
        d["R"] = R
        R0, R1 = R[0][:], R[1][:]
        R2 = R[2][:]
        COLS = per.tile([P, 8], f32, tag="COLS", bufs=1, name="COLS")
        V.memset(COLS[:], 0.0)
        d["COLS"] = COLS

        pos = per.tile([P, F], f32, tag="pos", bufs=1, name="pos")
        V.scalar_tensor_tensor(pos[:], mp[:], LN05, facc[:],
                               op0=Alu.is_ge, op1=Alu.max,
                               accum_out=COLS[:, 0:1])
        neg = per.tile([P, F], f32, tag="neg", bufs=1, name="neg")
        V.scalar_tensor_tensor(R0, mp[:], LN04, facc[:],
                               op0=Alu.is_lt, op1=Alu.subtract)
        V.tensor_scalar(neg[:], R0, 0.0, None, op0=Alu.max, op1=Alu.add,
                        accum_out=COLS[:, 1:2])

        # focal: fl = (0.5*pos - 0.75) * (pt-1)^2 * ln(pt)
        fl = per.tile([P, F], f32, tag="fl", bufs=1, name="fl")
        V.tensor_mul(R0, pos[:], CPt[:])
        V.scalar_tensor_tensor(R0, R0, 2.0, CPt[:],
                               op0=Alu.mult, op1=Alu.subtract)
        V.tensor_sub(R0, R0, pos[:])                 # pt - 1
        S.activation(R1, R0, Act.Ln, bias=1.0)       # ln(pt)
        S.activation(R2, R0, Act.Square)             # (1-pt)^2
        V.tensor_mul(R1, R1, R2)
        V.tensor_scalar(R0, pos[:], 0.5, 0.75, op0=Alu.mult, op1=Alu.subtract)
        V.tensor_mul(fl[:], R0, R1)
        V.scalar_tensor_tensor(R0, fl[:], 1.0, pos[:],
                               op0=Alu.mult, op1=Alu.mult,
                               accum_out=COLS[:, 2:3])
        ns = per.tile([P, F], f32, tag="ns", bufs=1, name="ns")
        V.tensor_mul(ns[:], fl[:], neg[:])
        d["pos"], d["ns"], d["fl"] = pos, ns, fl

    def epilogue2(b):
        d = st[b]
        M0, M1, M2, M3 = d["M"]
        BX1, BY1, BX2, BY2 = d["BX1"][:], d["BY1"][:], d["BX2"][:], d["BY2"][:]
        E = [per.tile([P, F], f16, tag=f"E{i}", bufs=1, name=f"E{i}")
             for i in range(5)]
        d["E"] = E
        R0, R1, R2, R3, R4 = [e[:] for e in E]
        V.tensor_max(R0, BX1, M0[:])
        V.tensor_tensor(R1, BX2, M2[:], op=Alu.min)
        V.tensor_sub(R1, R1, R0)
        V.tensor_scalar(R1, R1, 0.0, None, op0=Alu.max)   # dxc
        V.tensor_max(R0, BY1, M1[:])
        V.tensor_tensor(R2, BY2, M3[:], op=Alu.min)
        V.tensor_sub(R2, R2, R0)
        V.tensor_scalar(R2, R2, 0.0, None, op0=Alu.max)   # dyc
        V.tensor_mul(R1, R1, R2)                          # bint
        V.tensor_sub(R0, BX2, BX1)
        V.tensor_sub(R2, BY2, BY1)
        V.tensor_mul(R0, R0, R2)                          # areab
        V.tensor_sub(R2, M2[:], M0[:])
        V.tensor_sub(R3, M3[:], M1[:])
        V.tensor_mul(R2, R2, R3)                          # aream
        V.tensor_add(R0, R0, R2)
        V.tensor_sub(R0, R0, R1)                          # uni
        S.activation(R2, R0, Act.Ln, bias=CEPS[:])
        S.activation(R2, R2, Act.Exp, scale=-1.0)         # 1/(uni+eps)
        V.tensor_mul(R1, R1, R2)                          # ioub
        V.tensor_max(R2, BX2, M2[:])
        V.tensor_tensor(R3, BX1, M0[:], op=Alu.min)
        V.tensor_sub(R2, R2, R3)                          # dex
        V.tensor_max(R3, BY2, M3[:])
        V.tensor_tensor(R4, BY1, M1[:], op=Alu.min)
        V.tensor_sub(R3, R3, R4)                          # dey
        V.tensor_mul(R2, R2, R3)                          # enc
        S.activation(R3, R2, Act.Ln, bias=CEPS[:])
        S.activation(R3, R3, Act.Exp, scale=-1.0)         # 1/(enc+eps)
        V.tensor_sub(R2, R2, R0)                          # enc-uni
        V.tensor_mul(R2, R2, R3)
        V.tensor_sub(R2, R2, R1)                  # pb0 = (enc-uni)/(enc+e)-iou

    def epilogue3(b):
        d = st[b]
        M0, M1, M2, M3 = d["M"]
        BX1, BY1, BX2, BY2 = d["BX1"][:], d["BY1"][:], d["BX2"][:], d["BY2"][:]
        E = d["E"]
        R0, R1, R2, R3 = [e[:] for e in E[:4]]
        COLS, pos = d["COLS"], d["pos"]
        V.tensor_sub(R0, BX1, M0[:])
        S.activation(R0, R0, Act.Abs)
        V.tensor_sub(R1, BY1, M1[:])
        S.activation(R1, R1, Act.Abs)
        V.tensor_add(R0, R0, R1)
        V.tensor_sub(R1, BX2, M2[:])
        S.activation(R1, R1, Act.Abs)
        V.tensor_sub(R3, BY2, M3[:])
        S.activation(R3, R3, Act.Abs)
        V.tensor_add(R1, R1, R3)
        V.tensor_add(R0, R0, R1)                          # l1 sum
        V.scalar_tensor_tensor(R2, R0, 0.125, R2, op0=Alu.mult, op1=Alu.add)
        V.scalar_tensor_tensor(R3, R2, 1.0, pos[:],
                               op0=Alu.add, op1=Alu.mult,
                               accum_out=COLS[:, 3:4])

    def sums_kk(b):
        d = st[b]
        SUMS = sm.tile([P, 8], f32, name="SUMS")
        G.partition_all_reduce(SUMS[:], d["COLS"][:], P, Red.add)
        kk = per.tile([P, 1], f32, tag="kk", bufs=1, name="kk")
        npos = SUMS[:, 0:1]
        nneg = SUMS[:, 1:2]
        V.scalar_tensor_tensor(kk[:], npos, 3.0, nneg, op0=Alu.mult,
                               op1=Alu.min)
        k0 = sm.tile([P, 1], f32, name="k0")
        V.tensor_scalar(k0[:], nneg, 100.0, None, op0=Alu.min)
        zz = sm.tile([P, 1], f32, name="zz")
        V.tensor_scalar(zz[:], npos, 0.5, None, op0=Alu.is_lt)
        kd = sm.tile([P, 1], f32, name="kd")
        V.tensor_sub(kd[:], k0[:], kk[:])
        V.tensor_mul(kd[:], kd[:], zz[:])
        V.tensor_add(kk[:], kk[:], kd[:])
        d["kk"], d["SUMS"] = kk, SUMS
        lo = per.tile([P, 1], f32, tag="lo", bufs=1, name="lo")
        V.memset(lo[:], 0.0)
        d["lo"] = lo
        d["step"] = 2.0

    def bins_iter(b, it):
        d = st[b]
        ns, kk, lo = d["ns"], d["kk"], d["lo"]
        R5 = d["R"][0][:]
        step = d["step"]
        tau = sm.tile([P, 1], f32, name="tau")
        V.tensor_scalar(tau[:], lo[:], step, None, op0=Alu.add)
        cntc = sm.tile([P, 1], f32, name="cntc")
        V.tensor_scalar(R5, ns[:], tau[:], None, op0=Alu.is_gt,
                        op1=Alu.add, accum_out=cntc[:])
        cntb = sm.tile([P, 1], f32, name="cntb")
        G.partition_all_reduce(cntb[:], cntc[:], P, Red.add)
        gg = sm.tile([P, 1], f32, name="gg")
        V.tensor_tensor(gg[:], cntb[:], kk[:], op=Alu.is_ge)
        V.scalar_tensor_tensor(lo[:], gg[:], step, lo[:],
                               op0=Alu.mult, op1=Alu.add)
        d["step"] = step * 0.5

    def bins_final(b):
        d = st[b]
        ns, kk, lo, SUMS = d["ns"], d["kk"], d["lo"], d["SUMS"]
        R5 = d["R"][0][:]
        npos = SUMS[:, 0:1]
        pfs = SUMS[:, 2:3]
        pbs = SUMS[:, 3:4]
        cnt2 = sm.tile([P, 2], f32, name="cnt2")
        V.tensor_scalar(R5, ns[:], lo[:], None, op0=Alu.is_gt,
                        op1=Alu.add, accum_out=cnt2[:, 0:1])
        V.scalar_tensor_tensor(R5, ns[:], lo[:], ns[:],
                               op0=Alu.is_gt, op1=Alu.mult,
                               accum_out=cnt2[:, 1:2])
        C2 = sm.tile([P, 2], f32, name="C2")
        G.partition_all_reduce(C2[:], cnt2[:], P, Red.add)
        nf = C2[:, 0:1]
        sf = C2[:, 1:2]

        kmn = sm.tile([P, 1], f32, name="kmn")
        V.tensor_sub(kmn[:], kk[:], nf)
        V.tensor_mul(kmn[:], kmn[:], lo[:])
        cnum = sm.tile([P, 1], f32, name="cnum")
        V.tensor_add(cnum[:], pfs, sf)
        V.tensor_add(cnum[:], cnum[:], kmn[:])
        den = sm.tile([P, 1], f32, name="den")
        V.tensor_add(den[:], npos, kk[:])
        V.tensor_scalar(den[:], den[:], 1.0, None, op0=Alu.max)
        rden = sm.tile([P, 1], f32, name="rden")
        V.reciprocal(rden[:], den[:])
        confl = sm.tile([P, 1], f32, name="confl")
        V.tensor_mul(confl[:], cnum[:], rden[:])

        np1 = sm.tile([P, 1], f32, name="np1")
        V.tensor_scalar(np1[:], npos, 1.0, None, op0=Alu.max)
        rnp = sm.tile([P, 1], f32, name="rnp")
        V.reciprocal(rnp[:], np1[:])
        bl0 = sm.tile([P, 1], f32, name="bl0")
        V.tensor_mul(bl0[:], pbs, rnp[:])
        zp = sm.tile([P, 1], f32, name="zp")
        V.tensor_scalar(zp[:], npos, 0.5, None, op0=Alu.is_gt)
        bbl = sm.tile([P, 1], f32, name="bbl")
        V.tensor_mul(bbl[:], bl0[:], zp[:])

        V.tensor_add(OUT[0:1, 0:1], OUT[0:1, 0:1], confl[0:1, :])
        V.tensor_add(OUT[0:1, 1:2], OUT[0:1, 1:2], bbl[0:1, :])

    def pass1_all(b, tail_chunks=None):
        # pass1 groups; optional other-image tail chunks interleaved
        tail_chunks = list(tail_chunks or [])
        NG = T // 2
        ci = 0
        for g in range(NG):
            pass1_group(b, g)
            if g >= 1:
                packed_group(b, g - 1)
            if g >= 2 and ci < len(tail_chunks):
                tail_chunks[ci]()
                ci += 1
            if g == 8:
                # first half of RM8 (t=0..15) is final; start its bounce
                SY.dma_start(
                    sc1_d[b][: T * P * 4].rearrange(
                        "(t p j) -> p t j", p=P, t=T // 2),
                    st[b]["RM8"][:, : T * 4].rearrange(
                        "p (t j) -> p t j", t=T // 2))
            if g == 9:
                T8 = stage.tile([T, P * 8], f16, tag="BPS", bufs=1, name="T8")
                st[b]["T8"] = T8
                SY.dma_start(T8[0 : T // 2, :],
                             sc1_d[b][: T * P * 4].rearrange(
                                 "(t x) -> t x", t=T // 2))
        packed_group(b, NG - 1)
        SY.dma_start(
            sc1_d[b][T * P * 4 :].rearrange(
                "(t p j) -> p t j", p=P, t=T // 2),
            st[b]["RM8"][:, T * 4 :].rearrange(
                "p (t j) -> p t j", t=T // 2))
        SY.dma_start(st[b]["T8"][T // 2 : T, :],
                     sc1_d[b][T * P * 4 :].rearrange(
                         "(t x) -> t x", t=T // 2))
        while ci < len(tail_chunks):
            tail_chunks[ci]()
            ci += 1

    def tail_stages(b):
        chunks = [lambda: epilogue1(b), lambda: epilogue2(b),
                  lambda: epilogue3(b), lambda: sums_kk(b)]
        chunks += [(lambda it=it: bins_iter(b, it)) for it in range(N_SEARCH)]
        chunks += [lambda: bins_final(b)]
        return chunks

    # ================= schedule =================
    prep(0)
    anchor_casts()
    areaa_build()
    load_bp(0)
    pass1_all(0)
    late_setup()
    eq_pass(0)
    thr_a(0)
    tbdg_build(0)   # fills the sc2 bounce round trip
    thr_b(0)
    pass2(0)
    pass3(0)
    prep(1)
    # img1 pass1 interleaved with img0 tail; load_bp(1) overwrites the BX/BY
    # tiles img0's epilogue reads, so it must follow epilogue3(0)
    t0c = tail_stages(0)
    chunks = t0c[:3] + [lambda: load_bp(1)] + t0c[3:]
    during, after = chunks[:-3], chunks[-3:]
    pass1_all(1, tail_chunks=during)
    eq_pass(1)
    thr_a(1)
    after[0]()
    tbdg_build(1)
    after[1]()
    thr_b(1)
    after[2]()
    pass2(1)
    pass3(1)
    for c in tail_stages(1):
        c()

    SY.dma_start(out_d, OUT[0:1, :])


def build():
    import concourse.bacc as bacc
    import concourse.mybir as mybir
    import concourse.tile as tile

    f32 = mybir.dt.float32
    nc = bacc.Bacc("TRN2", target_bir_lowering=False, debug=False)
    bp_d = nc.dram_tensor("bp", [BI, A, 4], f32, kind="ExternalInput")
    cp_d = nc.dram_tensor("cp", [BI, A], f32, kind="ExternalInput")
    an_d = nc.dram_tensor("an", [A, 4], f32, kind="ExternalInput")
    tb_d = nc.dram_tensor("tb", [BI, T, 4], f32, kind="ExternalInput")
    out_d = nc.dram_tensor("out", [2], f32, kind="ExternalOutput")
    sc1_d = nc.dram_tensor("scratch1", [BI, T * P * 8], mybir.dt.float16)
    sc2_d = nc.dram_tensor("scratch2", [BI, T], f32)
    sc3_d = nc.dram_tensor("scratch3", [T], f32)
    sc4_d = nc.dram_tensor("scratch4", [P * 129], f32)
    sc5_d = nc.dram_tensor("scratch5", [P * 256 + 256], f32)
    sc6_d = nc.dram_tensor("scratch6", [256], f32)
    with tile.TileContext(nc) as tc:
        with ExitStack() as ctx:
            _emit(nc, tc, ctx, bp_d.ap(), cp_d.ap(), an_d.ap(), tb_d.ap(),
                  out_d.ap(), sc1_d.ap(), sc2_d.ap(), sc3_d.ap(), sc4_d.ap(),
                  sc5_d.ap(), sc6_d.ap())
    nc.compile()
    return nc


def kernel(bbox_pred, conf_pred, anchors, target_boxes):
    from concourse.bass_utils import run_bass_kernel_spmd

    bp = np.ascontiguousarray(np.asarray(bbox_pred, dtype=np.float32))
    cp = np.ascontiguousarray(np.asarray(conf_pred, dtype=np.float32))
    an = np.ascontiguousarray(np.asarray(anchors, dtype=np.float32))
    tb = np.ascontiguousarray(np.asarray(target_boxes, dtype=np.float32))

    nc = build()
    in_maps = []
    for k in range(NCORES):
        sl = slice(BI * k, BI * (k + 1))
        in_maps.append({"bp": bp[sl], "cp": cp[sl], "an": an, "tb": tb[sl]})
    trace = bool(int(os.environ.get("DETLOSS_TRACE", "0")))
    res = run_bass_kernel_spmd(nc, in_maps, list(range(NCORES)), trace=trace)
    partials = np.stack([res.results[k]["out"] for k in range(NCORES)])  # [8,2]
    conf = np.float32(partials[:, 0].sum() / 16.0)
    bbox = np.float32(partials[:, 1].sum() / 16.0)
    total = np.float32(conf + bbox)
    if trace:
        kernel.last_exec_time_ns = res.exec_time_ns
        kernel.last_trace = res.instructions_and_trace
    return (total, conf, bbox)


# revision 4
# speedup vs baseline: 1.1014x; 1.0205x over previous
"""DetectionLoss Trainium2 kernel, v2d.

Contract: kernel(**inputs) takes FULL inputs (bbox_pred [16,65536,4],
conf_pred [16,65536], anchors [65536,4], target_boxes [16,32,4]) and
returns the full output (total_loss, conf_loss, bbox_loss) as f32 scalars.

Sharding: data-parallel over batch. Core k processes images 2k, 2k+1 and
emits (sum conf_l, sum bbox_l); the host divides by 16 and sums.

v2d plan:
  - packed log-iou stored T-MAJOR; index-perturbation folded into the Ln
    scale f_t = exp((31-t)*EPSI).
  - interval algebra puts the per-target max on Scalar:
      dx = min(AX2-tx1, tw) - relu(AX1-tx1)
    with relu on S (bias = pre-negated target coord) and min as a
    dual-scalar 2x-mode tensor_scalar on V.
  - all bulk elementwise on V (GpSimd compute measured as a net loss:
    shared SBUF ports stall concurrent V 2x-mode ops ~1:1). GpSimd keeps
    only partition_all_reduce (280ns) for all cross-partition sums.
  - matched-box gather: eq blocks in (t,f) order, PE transposes + 16
    accumulating 128x128x256 fp16 matmuls per 64-f chunk against
    diagonal-mask rhs tiles (TBDG, built on S).
  - binary search 7 iters via partition_all_reduce.
  - cross-image software pipelining: img1's pass-1 groups are emitted
    between img0's tail stages (pass3/epilogue/binary search) so V chews
    intervals while img0's PE/S/DMA serial chains progress. IOU (bufs=1)
    hazard avoided by ordering img0's eq+pass2 before the interleave.
"""

import os
from contextlib import ExitStack

import numpy as np

P = 128          # SBUF partitions
F = 512          # anchors per partition row
T = 32           # targets per image
BI = 2           # images per core
A = P * F        # 65536 anchors
NCORES = 8

EPS = 1e-6
TINY = 1e-38
EPSI = float(2.0 ** -20)          # index-packing epsilon (log space)
DELTA = 2e-6                      # strictness margin for iou > 0.3
LN05 = float(np.log(0.5)) + 15.0 * EPSI
LN04 = float(np.log(0.4)) + 15.0 * EPSI
LN03D = float(np.log(0.3)) + DELTA
N_SEARCH = 5                      # binary-search iterations for kth value
F_T = [float(np.exp((31 - t) * EPSI)) for t in range(T)]


def _emit(nc, tc, ctx, bp_d, cp_d, an_d, tb_d, out_d, sc1_d, sc2_d, sc3_d,
          sc4_d, sc5_d, sc6_d, sc7_d):
    import concourse.bass as bass
    import concourse.bass_isa as bass_isa
    import concourse.mybir as mybir

    f32 = mybir.dt.float32
    f16 = mybir.dt.float16
    Alu = mybir.AluOpType
    Act = mybir.ActivationFunctionType
    Red = bass_isa.ReduceOp
    V = nc.vector
    S = nc.scalar
    G = nc.gpsimd
    PE = nc.tensor
    SY = nc.sync
    ts = bass.ts

    big = ctx.enter_context(tc.tile_pool(name="big", bufs=1))
    stage = ctx.enter_context(tc.tile_pool(name="stage", bufs=1))
    per = ctx.enter_context(tc.tile_pool(name="per", bufs=1))
    tp = ctx.enter_context(tc.tile_pool(name="tp", bufs=2))
    sm = ctx.enter_context(tc.tile_pool(name="sm", bufs=2))
    ps = ctx.enter_context(tc.tile_pool(name="ps", bufs=2, space="PSUM"))

    # ---- input loads first: smalls, then big chunked loads
    TBrow0 = sm.tile([1, T * 4], f32, bufs=2, name="TBrow0")
    SY.dma_start(TBrow0[:], tb_d[0].rearrange("t c -> (t c)")[None, :])
    EV0 = sm.tile([2, 64], f32, bufs=2, name="EV0")
    SY.dma_start(EV0[:].rearrange("p (g c) -> p g c", c=4),
                 tb_d[0].rearrange("(g two) c -> two g c", two=2))
    CPt0 = per.tile([P, F], f32, tag="CPt", bufs=2, name="CPt0")
    SY.dma_start(CPt0[:], cp_d[0].rearrange("(p f) -> p f", p=P))
    AXYS = stage.tile([P, F * 4], f32, tag="BPS")
    anv = an_d.rearrange("(p f) c -> p (f c)", p=P)
    for _q in range(8):
        SY.dma_start(AXYS[:, ts(_q, F // 2)], anv[:, ts(_q, F // 2)])
    BPS0 = stage.tile([P, F * 4], f32, tag="BPS", bufs=1, name="BPS0")
    bpv0 = bp_d[0].rearrange("(p f) c -> p (f c)", p=P)
    for _q in range(8):
        SY.dma_start(BPS0[:, ts(_q, F // 2)], bpv0[:, ts(_q, F // 2)])
    preload = {0: (BPS0, CPt0, TBrow0, EV0)}

    AXv = AXYS[:].rearrange("p (f c) -> p c f", c=4)
    AX1 = big.tile([P, F], f16)
    AY1 = big.tile([P, F], f16)
    AX2 = big.tile([P, F], f16)
    AY2 = big.tile([P, F], f16)

    def anchor_casts():
        for _q in range(8):
            _sl = slice(64 * _q, 64 * (_q + 1))
            V.tensor_copy(AX1[:, _sl], AXv[:, 0, _sl])
            V.tensor_copy(AY1[:, _sl], AXv[:, 1, _sl])
            V.tensor_copy(AX2[:, _sl], AXv[:, 2, _sl])
            V.tensor_copy(AY2[:, _sl], AXv[:, 3, _sl])
    AREAA = big.tile([P, F], f16)

    def areaa_build():
        aw0 = tp.tile([P, F], f16, tag="mx", bufs=3)
        ah0 = tp.tile([P, F], f16, tag="my", bufs=3)
        V.tensor_sub(aw0[:], AX2[:], AX1[:])
        V.tensor_sub(ah0[:], AY2[:], AY1[:])
        V.tensor_mul(AREAA[:], aw0[:], ah0[:])

    # per-target ln(0.3) + (31-t)*EPSI + DELTA on partitions 0..31
    L03row = sm.tile([1, T], f32)
    for t in range(T):
        V.memset(L03row[:, t : t + 1], LN03D + (31 - t) * EPSI)
    SY.dma_start(sc3_d, L03row[:])
    L03C = big.tile([T, 1], f32)
    SY.dma_start(L03C[:], sc3_d.rearrange("(t one) -> t one", one=1))

    ONES = big.tile([P, 1], f32)
    V.memset(ONES[:], 1.0)
    ONESROW = big.tile([1, P], f32)
    V.memset(ONESROW[:], 1.0)
    ONES32 = big.tile([T, P], f32)
    V.memset(ONES32[:], 1.0)
    # identity matrix via DRAM bounce: diag of [128,128] at stride 129
    ZED = stage.tile([P, P], f32, tag="ZED")
    V.memset(ZED[:], 0.0)
    SY.dma_start(sc4_d[: P * P].rearrange("(p f) -> p f", p=P), ZED[:])
    SY.dma_start(
        sc4_d[: P * 129].rearrange("(p f) -> p f", f=129)[:, 0:1], ONES[:])
    IDTV = stage.tile([P, P], f32, tag="IDTV", name="IDTV")
    SY.dma_start(IDTV[:], sc4_d[: P * P].rearrange("(p f) -> p f", p=P))
    IDTH = big.tile([P, P], f16)

    def late_setup():
        # IDTV bounce has long landed by now; no V head-block
        V.tensor_copy(IDTH[:], IDTV[:])

    # DIAG64 [P, 256] f32: D[p, c*64 + f] = (f == p % 64), via DRAM bounce.
    ZED2 = stage.tile([P, 256], f32, tag="ZED2")
    V.memset(ZED2[:], 0.0)
    SY.dma_start(sc5_d[: P * 256].rearrange("(p f) -> p f", p=P), ZED2[:])
    for half in range(2):
        for c in range(4):
            base = half * 64 * 256 + c * 64
            SY.dma_start(
                sc5_d[base : base + 64 * 257]
                .rearrange("(r x) -> r x", x=257)[:, 0:1],
                ONES[0:64, :])
    DIAG64 = big.tile([P, 256], f32)
    SY.dma_start(DIAG64[:], sc5_d[: P * 256].rearrange("(p f) -> p f", p=P))

    # lhsT for the 2-row broadcast matmul (TBALL build), via DRAM bounce
    ZROW = sm.tile([1, 256], f32)
    V.memset(ZROW[:, 0:64], 1.0)
    V.memset(ZROW[:, 64:192], 0.0)
    V.memset(ZROW[:, 192:256], 1.0)
    SY.dma_start(sc6_d, ZROW[:])
    LH2 = big.tile([2, P], f32)
    SY.dma_start(LH2[:], sc6_d.rearrange("(p f) -> p f", p=2))

    ZEDH = stage.tile([P, 256], f16, tag="ZED2", name="ZEDH")
    V.memset(ZEDH[:], 0.0)
    _f7 = sc7_d[: 16 * 32768].rearrange("(p x) -> p x", x=4096)
    for _g in range(16):
        SY.dma_start(_f7[:, 256 * _g : 256 * (_g + 1)], ZEDH[:])
    CTINY = big.tile([P, 1], f32)
    V.memset(CTINY[:], TINY)
    CEPS = big.tile([P, 1], f32)
    V.memset(CEPS[:], EPS)
    OUT = big.tile([1, 2], f32)
    V.memset(OUT[:], 0.0)

    IOU = big.tile([P, T * F], f16)     # packed log-iou (fp16), t-MAJOR

    def iou_t(t):
        return IOU[:, t * F : (t + 1) * F]

    st = [dict() for _ in range(BI)]

    # ================= per-image stages =================

    def prep(b):
        d = st[b]
        if b in preload:
            BPS, CPt, TBrow, EV = preload[b]
        else:
            BPS = stage.tile([P, F * 4], f32, tag="BPS", bufs=1, name="BPS")
            bpv = bp_d[b].rearrange("(p f) c -> p (f c)", p=P)
            for _q in range(4):
                SY.dma_start(BPS[:, ts(_q, F)], bpv[:, ts(_q, F)])
            CPt = per.tile([P, F], f32, tag="CPt", bufs=2, name="CPt")
            SY.dma_start(CPt[:], cp_d[b].rearrange("(p f) -> p f", p=P))
            TBrow = sm.tile([1, T * 4], f32, bufs=2, name="TBrow")
            SY.dma_start(TBrow[:], tb_d[b].rearrange("t c -> (t c)")[None, :])
            EV = sm.tile([2, 64], f32, bufs=2, name="EV")
            SY.dma_start(EV[:].rearrange("p (g c) -> p g c", c=4),
                         tb_d[b].rearrange("(g two) c -> two g c", two=2))
        d["BPS"] = BPS
        d["CPt"] = CPt
        d["EV"] = EV
        TBrowV = sm.tile([1, T * 4], f32, bufs=2, name="TBrowV")
        V.tensor_copy(TBrowV[:], TBrow[:])
        TBB = per.tile([P, T * 4], f32, tag="TBB", bufs=1, name="TBB")
        tbb_ps = ps.tile([P, T * 4], f32, tag="bc_ps", bufs=1, name="tbb_ps")
        PE.matmul(tbb_ps[:], ONESROW[:], TBrowV[:], start=True, stop=True)
        S.copy(TBB[:], tbb_ps[:])
        d["TBB"] = TBB
        NTB = per.tile([P, T * 4], f32, tag="NTB", bufs=1, name="NTB")
        V.tensor_scalar(NTB[:], TBB[:], -1.0, None, op0=Alu.mult)
        d["NTB"] = NTB
        TBv = TBB[:].rearrange("p (t c) -> p c t", c=4)
        TW = per.tile([P, T], f32, tag="TW", bufs=1, name="TW")
        TH = per.tile([P, T], f32, tag="TH", bufs=1, name="TH")
        ABE = per.tile([P, T], f32, tag="ABE", bufs=1, name="ABE")
        V.tensor_sub(TW[:], TBv[:, 2], TBv[:, 0])
        V.tensor_sub(TH[:], TBv[:, 3], TBv[:, 1])
        V.tensor_mul(ABE[:], TW[:], TH[:])
        V.tensor_scalar(ABE[:], ABE[:], EPS, None, op0=Alu.add)
        d["TW"], d["TH"], d["ABE"] = TW, TH, ABE

        d["mp"] = per.tile([P, F], f16, tag="mp", bufs=2, name="mp")
        d["RM8"] = per.tile([P, T * 8], f16, tag="RM8", bufs=1, name="RM8")
        d["groups"] = {}

    def load_bp(b):
        d = st[b]
        BPv = d["BPS"][:].rearrange("p (f c) -> p c f", c=4)
        BX1 = per.tile([P, F], f16, tag="BX1", bufs=1, name="BX1")
        BY1 = per.tile([P, F], f16, tag="BY1", bufs=1, name="BY1")
        BX2 = per.tile([P, F], f16, tag="BX2", bufs=1, name="BX2")
        BY2 = per.tile([P, F], f16, tag="BY2", bufs=1, name="BY2")
        S.copy(BX1[:], BPv[:, 0])
        S.copy(BY1[:], BPv[:, 1])
        S.copy(BX2[:], BPv[:, 2])
        S.copy(BY2[:], BPv[:, 3])
        d["BX1"], d["BY1"], d["BX2"], d["BY2"] = BX1, BY1, BX2, BY2

    def ncol(d, tt, c):
        return d["NTB"][:, 4 * tt + c : 4 * tt + c + 1]

    def pass1_group(b, g):
        d = st[b]
        t0, t1 = 2 * g, 2 * g + 1
        INTER2 = tp.tile([P, 2 * F], f16, tag="INTER2", bufs=3,
                         name=f"INTER2_{b}_{g}")
        UNION2 = tp.tile([P, 2 * F], f16, tag="UNION2", bufs=3,
                         name=f"UNION2_{b}_{g}")
        for k, t in enumerate((t0, t1)):
            # x: dx = min(AX2 - tx1, tw) - relu(AX1 - tx1)   (all fp16)
            rx = tp.tile([P, F], f16, tag="rx", bufs=3, name="rx")
            S.activation(rx[:], AX1[:], Act.Relu, bias=ncol(d, t, 0))
            mx = tp.tile([P, F], f16, tag="mx", bufs=3, name="mx")
            V.tensor_scalar(mx[:], AX2[:], ncol(d, t, 0),
                            d["TW"][:, t : t + 1], op0=Alu.add, op1=Alu.min)
            V.tensor_sub(mx[:], mx[:], rx[:])                    # dx
            # y: dy = min(AY2 - ty1, th) - relu(AY1 - ty1)
            ry = tp.tile([P, F], f16, tag="ry", bufs=3, name="ry")
            S.activation(ry[:], AY1[:], Act.Relu, bias=ncol(d, t, 1))
            my = tp.tile([P, F], f16, tag="my", bufs=3, name="my")
            V.tensor_scalar(my[:], AY2[:], ncol(d, t, 1),
                            d["TH"][:, t : t + 1], op0=Alu.add, op1=Alu.min)
            V.tensor_sub(my[:], my[:], ry[:])                    # dy
            V.tensor_scalar(my[:], my[:], 0.0, F_T[t],
                            op0=Alu.max, op1=Alu.mult)           # dyr (inpl)
            isl = INTER2[:, k * F : (k + 1) * F]
            V.scalar_tensor_tensor(isl, mx[:], 0.0, my[:],
                                   op0=Alu.max, op1=Alu.mult)    # inter
            V.tensor_sub(UNION2[:, k * F : (k + 1) * F], AREAA[:], isl)
        d["groups"][g] = (INTER2, UNION2)

    def packed_group(b, g):
        # emitted one group behind pass1_group so the S queue has the next
        # group's relus ahead of these Lns
        d = st[b]
        t0, t1 = 2 * g, 2 * g + 1
        INTER2, UNION2 = d["groups"].pop(g)
        ABE = d["ABE"]
        LI2 = tp.tile([P, 2 * F], f16, tag="LI2", bufs=3, name="LI2")
        LU2 = tp.tile([P, 2 * F], f16, tag="LU2", bufs=3, name="LU2")
        S.activation(LI2[:], INTER2[:], Act.Ln, bias=CTINY[:])
        S.activation(LU2[:, 0:F], UNION2[:, 0:F], Act.Ln,
                     bias=ABE[:, t0 : t0 + 1])
        S.activation(LU2[:, F : 2 * F], UNION2[:, F : 2 * F], Act.Ln,
                     bias=ABE[:, t1 : t1 + 1])
        V.tensor_tensor(IOU[:, t0 * F : (t1 + 1) * F], LI2[:],
                        LU2[:], op=Alu.subtract)
        V.max(d["RM8"][:, ts(t0, 8)], iou_t(t0))
        V.max(d["RM8"][:, ts(t1, 8)], iou_t(t1))
        mp = d["mp"]
        if g == 0:
            S.copy(mp[:], iou_t(0))
            V.tensor_max(mp[:], mp[:], iou_t(1))
        else:
            V.tensor_max(mp[:], mp[:], iou_t(t0))
            V.tensor_max(mp[:], mp[:], iou_t(t1))

    def thr_a(b):
        d = st[b]
        T8 = d["T8"]
        G8 = sm.tile([T, 8], f16, name="G8")
        V.max(G8[:], T8[:])
        G8F = sm.tile([T, 8], f32, name="G8F")
        S.copy(G8F[:], G8[:])
        mx8 = sm.tile([T, 1], f32, name="mx8")
        V.tensor_max(mx8[:], G8F[:, 2:3], L03C[:])
        thr = sm.tile([T, 1], f32, name="thr")
        V.tensor_tensor(thr[:], mx8[:], G8F[:, 0:1], op=Alu.min)
        # D32 = diag(thr): THR[i, j] = sum_p ones32[p, i] * D32[p, j] = thr[j]
        D32 = sm.tile([T, T], f32, name="D32")
        V.tensor_tensor(D32[:], IDTV[0:T, 0:T], thr[:].broadcast_to([T, T]),
                        op=Alu.mult)
        d["D32"] = D32

    def thr_b(b):
        d = st[b]
        THR = per.tile([P, T], f32, tag="THR", bufs=1, name="THR")
        thr_ps = ps.tile([P, T], f32, tag="bc_ps", bufs=1, name="thr_ps")
        PE.matmul(thr_ps[:], ONES32[:], d["D32"][:], start=True, stop=True)
        S.copy(THR[:], thr_ps[:])
        d["THR"] = THR

    def eq_pass(b):
        d = st[b]
        mp = d["mp"]
        IOU3 = IOU[:].rearrange("p (t f) -> p t f", t=T)
        eqbs = []
        for q in range(8):
            EQB = per.tile([P, T * 64], f16, tag="EQB", bufs=8, name="EQB")
            V.tensor_tensor(
                EQB[:].rearrange("p (t f) -> p t f", t=T),
                IOU3[:, :, q * 64 : (q + 1) * 64],
                mp[:, q * 64 : (q + 1) * 64]
                .rearrange("p (one f) -> p one f", one=1)
                .broadcast_to([P, T, 64]),
                op=Alu.is_equal)
            eqbs.append(EQB)
        d["eqbs"] = eqbs

    def pass2(b):
        d = st[b]
        THR = d["THR"]
        facc = per.tile([P, F], f32, tag="facc", bufs=1, name="facc")
        V.tensor_scalar(facc[:], iou_t(0), THR[:, 0:1], None, op0=Alu.is_ge)
        for t in range(1, T):
            V.scalar_tensor_tensor(facc[:], iou_t(t), THR[:, t : t + 1],
                                   facc[:], op0=Alu.is_ge, op1=Alu.max)
        d["facc"] = facc

    def tbdg_build(b):
        d = st[b]
        # TBALL [P, 64]: row-block h=p//64 holds tb[2g+h, c] at col g*4+c
        EV = d["EV"]
        EVv = sm.tile([2, 64], f32, bufs=2, name="EVv")
        V.tensor_copy(EVv[:], EV[:])
        tball_ps = ps.tile([P, 64], f32, tag="bc_ps", bufs=1, name="tball_ps")
        PE.matmul(tball_ps[:], LH2[:], EVv[:], start=True, stop=True)
        TBALL = per.tile([P, 64], f16, tag="TBALL", bufs=2, name="TBALL")
        S.copy(TBALL[:], tball_ps[:])
        # build TBDG in DRAM (p-major: F[p*4096 + g*256 + j]): the zero
        # background persists; each image rewrites only the diagonals.
        # diag entries for (half, c): offset half*64*4096 + c*64 + r*4097
        # + g*256  -> view [r:64 (4097), g:16 (256)], matching the SBUF
        # source TBALL[64h:64h+64, c::4] ([64 partitions, 16 cols]).
        TBV = TBALL[:].rearrange("p (g c) -> p c g", c=4)
        for half in range(2):
            for c in range(4):
                off = half * 64 * 4096 + c * 64
                SY.dma_start(
                    sc7_d[off : off + 64 * 4097]
                    .rearrange("(r x) -> r x", x=4097)[:, 0:4096]
                    .rearrange("r (g y) -> r g y", y=256)[:, :, 0:1],
                    TBV[64 * half : 64 * half + 64, c, :, None])
        TBDG = per.tile([P, 16 * 256], f16, tag="TBDG", bufs=1, name="TBDG")
        SY.dma_start(TBDG[:], sc7_d[: 16 * 32768].rearrange("(p x) -> p x", x=4096))
        d["TBDG"] = TBDG

    def pass3(b):
        # PE + S only: transposes, gather matmuls, M copies
        d = st[b]
        TBDG = d["TBDG"]
        M0 = per.tile([P, F], f16, tag="M0", bufs=1, name="M0")
        M1 = per.tile([P, F], f16, tag="M1", bufs=1, name="M1")
        M2 = per.tile([P, F], f16, tag="M2", bufs=1, name="M2")
        M3 = per.tile([P, F], f16, tag="M3", bufs=1, name="M3")
        for q in range(8):
            EQB = d["eqbs"][q]
            mm_ps = ps.tile([P, 256], f32, tag="mm_ps", bufs=2, name="mm_ps")
            for h in range(4):
                oh_ps = ps.tile([P, 512], f16, tag="oh_ps", bufs=2,
                                name="oh_ps")
                for j in range(4):
                    PE.transpose(oh_ps[:, ts(j, P)],
                                 EQB[:, ts(4 * h + j, P)], IDTH[:])
                OH = per.tile([P, 512], f16, tag="OH", bufs=2, name="OH")
                S.copy(OH[:], oh_ps[:])
                for j in range(4):
                    g = 4 * h + j
                    PE.matmul(mm_ps[:], OH[:, ts(j, P)],
                              TBDG[:, g * 256 : (g + 1) * 256],
                              start=(g == 0), stop=(g == 15))
            S.copy(M0[:, ts(q, 64)], mm_ps[:, 0:64])
            S.copy(M1[:, ts(q, 64)], mm_ps[:, 64:128])
            S.copy(M2[:, ts(q, 64)], mm_ps[:, 128:192])
            S.copy(M3[:, ts(q, 64)], mm_ps[:, 192:256])
        d["M"] = (M0, M1, M2, M3)

    def epilogue1(b):
        d = st[b]
        mp, facc, CPt = d["mp"], d["facc"], d["CPt"]
        R = [per.tile([P, F], f32, tag=f"R{i}", bufs=1, name=f"R{i}")
             for i in range(4)]
        d["R"] = R
        R0, R1 = R[0][:], R[1][:]
        R2 = R[2][:]
        COLS = per.tile([P, 8], f32, tag="COLS", bufs=1, name="COLS")
        V.memset(COLS[:], 0.0)
        d["COLS"] = COLS

        pos = per.tile([P, F], f32, tag="pos", bufs=1, name="pos")
        V.scalar_tensor_tensor(pos[:], mp[:], LN05, facc[:],
                               op0=Alu.is_ge, op1=Alu.max,
                               accum_out=COLS[:, 0:1])
        neg = per.tile([P, F], f32, tag="neg", bufs=1, name="neg")
        V.scalar_tensor_tensor(R0, mp[:], LN04, facc[:],
                               op0=Alu.is_lt, op1=Alu.subtract)
        V.tensor_scalar(neg[:], R0, 0.0, None, op0=Alu.max, op1=Alu.add,
                        accum_out=COLS[:, 1:2])

        # focal: fl = (0.5*pos - 0.75) * (pt-1)^2 * ln(pt)
        fl = per.tile([P, F], f32, tag="fl", bufs=1, name="fl")
        V.tensor_mul(R0, pos[:], CPt[:])
        V.scalar_tensor_tensor(R0, R0, 2.0, CPt[:],
                               op0=Alu.mult, op1=Alu.subtract)
        V.tensor_sub(R0, R0, pos[:])                 # pt - 1
        S.activation(R1, R0, Act.Ln, bias=1.0)       # ln(pt)
        S.activation(R2, R0, Act.Square)             # (1-pt)^2
        V.tensor_mul(R1, R1, R2)
        V.tensor_scalar(R0, pos[:], 0.5, 0.75, op0=Alu.mult, op1=Alu.subtract)
        V.tensor_mul(fl[:], R0, R1)
        V.scalar_tensor_tensor(R0, fl[:], 1.0, pos[:],
                               op0=Alu.mult, op1=Alu.mult,
                               accum_out=COLS[:, 2:3])
        ns = per.tile([P, F], f32, tag="ns", bufs=1, name="ns")
        V.tensor_mul(ns[:], fl[:], neg[:])
        d["pos"], d["ns"], d["fl"] = pos, ns, fl

    def epilogue2(b):
        d = st[b]
        M0, M1, M2, M3 = d["M"]
        BX1, BY1, BX2, BY2 = d["BX1"][:], d["BY1"][:], d["BX2"][:], d["BY2"][:]
        E = [per.tile([P, F], f16, tag=f"E{i}", bufs=1, name=f"E{i}")
             for i in range(5)]
        d["E"] = E
        R0, R1, R2, R3, R4 = [e[:] for e in E]
        V.tensor_max(R0, BX1, M0[:])
        V.tensor_tensor(R1, BX2, M2[:], op=Alu.min)
        V.tensor_sub(R1, R1, R0)
        V.tensor_scalar(R1, R1, 0.0, None, op0=Alu.max)   # dxc
        V.tensor_max(R0, BY1, M1[:])
        V.tensor_tensor(R2, BY2, M3[:], op=Alu.min)
        V.tensor_sub(R2, R2, R0)
        V.tensor_scalar(R2, R2, 0.0, None, op0=Alu.max)   # dyc
        V.tensor_mul(R1, R1, R2)                          # bint
        V.tensor_sub(R0, BX2, BX1)
        V.tensor_sub(R2, BY2, BY1)
        V.tensor_mul(R0, R0, R2)                          # areab
        V.tensor_sub(R2, M2[:], M0[:])
        V.tensor_sub(R3, M3[:], M1[:])
        V.tensor_mul(R2, R2, R3)                          # aream
        V.tensor_add(R0, R0, R2)
        V.tensor_sub(R0, R0, R1)                          # uni
        S.activation(R2, R0, Act.Ln, bias=CEPS[:])
        S.activation(R2, R2, Act.Exp, scale=-1.0)         # 1/(uni+eps)
        V.tensor_mul(R1, R1, R2)                          # ioub
        V.tensor_max(R2, BX2, M2[:])
        V.tensor_tensor(R3, BX1, M0[:], op=Alu.min)
        V.tensor_sub(R2, R2, R3)                          # dex
        V.tensor_max(R3, BY2, M3[:])
        V.tensor_tensor(R4, BY1, M1[:], op=Alu.min)
        V.tensor_sub(R3, R3, R4)                          # dey
        V.tensor_mul(R2, R2, R3)                          # enc
        S.activation(R3, R2, Act.Ln, bias=CEPS[:])
        S.activation(R3, R3, Act.Exp, scale=-1.0)         # 1/(enc+eps)
        V.tensor_sub(R2, R2, R0)                          # enc-uni
        V.tensor_mul(R2, R2, R3)
        V.tensor_sub(R2, R2, R1)                  # pb0 = (enc-uni)/(enc+e)-iou

    def epilogue3(b):
        d = st[b]
        M0, M1, M2, M3 = d["M"]
        BX1, BY1, BX2, BY2 = d["BX1"][:], d["BY1"][:], d["BX2"][:], d["BY2"][:]
        E = d["E"]
        R0, R1, R2, R3 = [e[:] for e in E[:4]]
        COLS, pos = d["COLS"], d["pos"]
        V.tensor_sub(R0, BX1, M0[:])
        S.activation(R0, R0, Act.Abs)
        V.tensor_sub(R1, BY1, M1[:])
        S.activation(R1, R1, Act.Abs)
        V.tensor_add(R0, R0, R1)
        V.tensor_sub(R1, BX2, M2[:])
        S.activation(R1, R1, Act.Abs)
        V.tensor_sub(R3, BY2, M3[:])
        S.activation(R3, R3, Act.Abs)
        V.tensor_add(R1, R1, R3)
        V.tensor_add(R0, R0, R1)                          # l1 sum
        V.scalar_tensor_tensor(R2, R0, 0.125, R2, op0=Alu.mult, op1=Alu.add)
        V.scalar_tensor_tensor(R3, R2, 1.0, pos[:],
                               op0=Alu.add, op1=Alu.mult,
                               accum_out=COLS[:, 3:4])

    def sums_kk(b):
        d = st[b]
        SUMS = sm.tile([P, 8], f32, name="SUMS")
        G.partition_all_reduce(SUMS[:], d["COLS"][:], P, Red.add)
        kk = per.tile([P, 1], f32, tag="kk", bufs=1, name="kk")
        npos = SUMS[:, 0:1]
        nneg = SUMS[:, 1:2]
        V.scalar_tensor_tensor(kk[:], npos, 3.0, nneg, op0=Alu.mult,
                               op1=Alu.min)
        k0 = sm.tile([P, 1], f32, name="k0")
        V.tensor_scalar(k0[:], nneg, 100.0, None, op0=Alu.min)
        zz = sm.tile([P, 1], f32, name="zz")
        V.tensor_scalar(zz[:], npos, 0.5, None, op0=Alu.is_lt)
        kd = sm.tile([P, 1], f32, name="kd")
        V.tensor_sub(kd[:], k0[:], kk[:])
        V.tensor_mul(kd[:], kd[:], zz[:])
        V.tensor_add(kk[:], kk[:], kd[:])
        d["kk"], d["SUMS"] = kk, SUMS
        lo = per.tile([P, 1], f32, tag="lo", bufs=1, name="lo")
        V.memset(lo[:], 0.0)
        d["lo"] = lo
        d["step"] = 2.0

    def bins_iter(b, it):
        d = st[b]
        ns, kk, lo = d["ns"], d["kk"], d["lo"]
        R5 = d["R"][0][:]
        step = d["step"]
        tau = sm.tile([P, 1], f32, name="tau")
        V.tensor_scalar(tau[:], lo[:], step, None, op0=Alu.add)
        cntc = sm.tile([P, 1], f32, name="cntc")
        V.tensor_scalar(R5, ns[:], tau[:], None, op0=Alu.is_gt,
                        op1=Alu.add, accum_out=cntc[:])
        cntb = sm.tile([P, 1], f32, name="cntb")
        G.partition_all_reduce(cntb[:], cntc[:], P, Red.add)
        gg = sm.tile([P, 1], f32, name="gg")
        V.tensor_tensor(gg[:], cntb[:], kk[:], op=Alu.is_ge)
        V.scalar_tensor_tensor(lo[:], gg[:], step, lo[:],
                               op0=Alu.mult, op1=Alu.add)
        d["step"] = step * 0.5

    def bins_final(b):
        d = st[b]
        ns, kk, lo, SUMS = d["ns"], d["kk"], d["lo"], d["SUMS"]
        R5 = d["R"][0][:]
        npos = SUMS[:, 0:1]
        pfs = SUMS[:, 2:3]
        pbs = SUMS[:, 3:4]
        cnt2 = sm.tile([P, 2], f32, name="cnt2")
        V.tensor_scalar(R5, ns[:], lo[:], None, op0=Alu.is_gt,
                        op1=Alu.add, accum_out=cnt2[:, 0:1])
        V.scalar_tensor_tensor(R5, ns[:], lo[:], ns[:],
                               op0=Alu.is_gt, op1=Alu.mult,
                               accum_out=cnt2[:, 1:2])
        C2 = sm.tile([P, 2], f32, name="C2")
        G.partition_all_reduce(C2[:], cnt2[:], P, Red.add)
        nf = C2[:, 0:1]
        sf = C2[:, 1:2]

        kmn = sm.tile([P, 1], f32, name="kmn")
        V.tensor_sub(kmn[:], kk[:], nf)
        V.tensor_mul(kmn[:], kmn[:], lo[:])
        cnum = sm.tile([P, 1], f32, name="cnum")
        V.tensor_add(cnum[:], pfs, sf)
        V.tensor_add(cnum[:], cnum[:], kmn[:])
        den = sm.tile([P, 1], f32, name="den")
        V.tensor_add(den[:], npos, kk[:])
        V.tensor_scalar(den[:], den[:], 1.0, None, op0=Alu.max)
        rden = sm.tile([P, 1], f32, name="rden")
        V.reciprocal(rden[:], den[:])
        confl = sm.tile([P, 1], f32, name="confl")
        V.tensor_mul(confl[:], cnum[:], rden[:])

        np1 = sm.tile([P, 1], f32, name="np1")
        V.tensor_scalar(np1[:], npos, 1.0, None, op0=Alu.max)
        rnp = sm.tile([P, 1], f32, name="rnp")
        V.reciprocal(rnp[:], np1[:])
        bl0 = sm.tile([P, 1], f32, name="bl0")
        V.tensor_mul(bl0[:], pbs, rnp[:])
        zp = sm.tile([P, 1], f32, name="zp")
        V.tensor_scalar(zp[:], npos, 0.5, None, op0=Alu.is_gt)
        bbl = sm.tile([P, 1], f32, name="bbl")
        V.tensor_mul(bbl[:], bl0[:], zp[:])

        V.tensor_add(OUT[0:1, 0:1], OUT[0:1, 0:1], confl[0:1, :])
        V.tensor_add(OUT[0:1, 1:2], OUT[0:1, 1:2], bbl[0:1, :])

    def pass1_all(b, tail_chunks=None):
        # pass1 groups; optional other-image tail chunks interleaved
        tail_chunks = list(tail_chunks or [])
        NG = T // 2
        ci = 0
        for g in range(NG):
            pass1_group(b, g)
            if g >= 1:
                packed_group(b, g - 1)
            if g >= 2 and ci < len(tail_chunks):
                tail_chunks[ci]()
                ci += 1
            if g == 8:
                # first half of RM8 (t=0..15) is final; start its bounce
                SY.dma_start(
                    sc1_d[b][: T * P * 4].rearrange(
                        "(t p j) -> p t j", p=P, t=T // 2),
                    st[b]["RM8"][:, : T * 4].rearrange(
                        "p (t j) -> p t j", t=T // 2))
            if g == 9:
                T8 = stage.tile([T, P * 8], f16, tag="BPS", bufs=1, name="T8")
                st[b]["T8"] = T8
                SY.dma_start(T8[0 : T // 2, :],
                             sc1_d[b][: T * P * 4].rearrange(
                                 "(t x) -> t x", t=T // 2))
        packed_group(b, NG - 1)
        SY.dma_start(
            sc1_d[b][T * P * 4 :].rearrange(
                "(t p j) -> p t j", p=P, t=T // 2),
            st[b]["RM8"][:, T * 4 :].rearrange(
                "p (t j) -> p t j", t=T // 2))
        SY.dma_start(st[b]["T8"][T // 2 : T, :],
                     sc1_d[b][T * P * 4 :].rearrange(
                         "(t x) -> t x", t=T // 2))
        while ci < len(tail_chunks):
            tail_chunks[ci]()
            ci += 1

    def tail_stages(b):
        chunks = [lambda: epilogue1(b), lambda: epilogue2(b),
                  lambda: epilogue3(b), lambda: sums_kk(b)]
        chunks += [(lambda it=it: bins_iter(b, it)) for it in range(N_SEARCH)]
        chunks += [lambda: bins_final(b)]
        return chunks

    # ================= schedule =================
    prep(0)
    anchor_casts()
    areaa_build()
    load_bp(0)
    pass1_all(0)
    late_setup()
    eq_pass(0)
    thr_a(0)
    tbdg_build(0)   # fills the sc2 bounce round trip
    thr_b(0)
    pass2(0)
    pass3(0)
    prep(1)
    # img1 pass1 interleaved with img0 tail; load_bp(1) overwrites the BX/BY
    # tiles img0's epilogue reads, so it must follow epilogue3(0)
    t0c = tail_stages(0)
    chunks = t0c[:3] + [lambda: load_bp(1)] + t0c[3:]
    during, after = chunks[:-3], chunks[-3:]
    pass1_all(1, tail_chunks=during)
    eq_pass(1)
    thr_a(1)
    after[0]()
    tbdg_build(1)
    after[1]()
    thr_b(1)
    after[2]()
    pass2(1)
    pass3(1)
    for c in tail_stages(1):
        c()

    SY.dma_start(out_d, OUT[0:1, :])


def build():
    import concourse.bacc as bacc
    import concourse.mybir as mybir
    import concourse.tile as tile

    f32 = mybir.dt.float32
    nc = bacc.Bacc("TRN2", target_bir_lowering=False, debug=False)
    bp_d = nc.dram_tensor("bp", [BI, A, 4], f32, kind="ExternalInput")
    cp_d = nc.dram_tensor("cp", [BI, A], f32, kind="ExternalInput")
    an_d = nc.dram_tensor("an", [A, 4], f32, kind="ExternalInput")
    tb_d = nc.dram_tensor("tb", [BI, T, 4], f32, kind="ExternalInput")
    out_d = nc.dram_tensor("out", [2], f32, kind="ExternalOutput")
    sc1_d = nc.dram_tensor("scratch1", [BI, T * P * 8], mybir.dt.float16)
    sc2_d = nc.dram_tensor("scratch2", [BI, T], f32)
    sc3_d = nc.dram_tensor("scratch3", [T], f32)
    sc4_d = nc.dram_tensor("scratch4", [P * 129], f32)
    sc5_d = nc.dram_tensor("scratch5", [P * 256 + 256], f32)
    sc6_d = nc.dram_tensor("scratch6", [256], f32)
    sc7_d = nc.dram_tensor("scratch7", [16 * 32768 + 256], mybir.dt.float16)
    with tile.TileContext(nc) as tc:
        with ExitStack() as ctx:
            _emit(nc, tc, ctx, bp_d.ap(), cp_d.ap(), an_d.ap(), tb_d.ap(),
                  out_d.ap(), sc1_d.ap(), sc2_d.ap(), sc3_d.ap(), sc4_d.ap(),
                  sc5_d.ap(), sc6_d.ap(), sc7_d.ap())
    nc.compile()
    return nc


def kernel(bbox_pred, conf_pred, anchors, target_boxes):
    from concourse.bass_utils import run_bass_kernel_spmd

    bp = np.ascontiguousarray(np.asarray(bbox_pred, dtype=np.float32))
    cp = np.ascontiguousarray(np.asarray(conf_pred, dtype=np.float32))
    an = np.ascontiguousarray(np.asarray(anchors, dtype=np.float32))
    tb = np.ascontiguousarray(np.asarray(target_boxes, dtype=np.float32))

    nc = build()
    in_maps = []
    for k in range(NCORES):
        sl = slice(BI * k, BI * (k + 1))
        in_maps.append({"bp": bp[sl], "cp": cp[sl], "an": an, "tb": tb[sl]})
    trace = bool(int(os.environ.get("DETLOSS_TRACE", "0")))
    res = run_bass_kernel_spmd(nc, in_maps, list(range(NCORES)), trace=trace)
    partials = np.stack([res.results[k]["out"] for k in range(NCORES)])  # [8,2]
    conf = np.float32(partials[:, 0].sum() / 16.0)
    bbox = np.float32(partials[:, 1].sum() / 16.0)
    total = np.float32(conf + bbox)
    if trace:
        kernel.last_exec_time_ns = res.exec_time_ns
        kernel.last_trace = res.instructions_and_trace
    return (total, conf, bbox)


# revision 5
# speedup vs baseline: 1.1151x; 1.0125x over previous
"""DetectionLoss Trainium2 kernel, v2d.

Contract: kernel(**inputs) takes FULL inputs (bbox_pred [16,65536,4],
conf_pred [16,65536], anchors [65536,4], target_boxes [16,32,4]) and
returns the full output (total_loss, conf_loss, bbox_loss) as f32 scalars.

Sharding: data-parallel over batch. Core k processes images 2k, 2k+1 and
emits (sum conf_l, sum bbox_l); the host divides by 16 and sums.

v2d plan:
  - packed log-iou stored T-MAJOR; index-perturbation folded into the Ln
    scale f_t = exp((31-t)*EPSI).
  - interval algebra puts the per-target max on Scalar:
      dx = min(AX2-tx1, tw) - relu(AX1-tx1)
    with relu on S (bias = pre-negated target coord) and min as a
    dual-scalar 2x-mode tensor_scalar on V.
  - all bulk elementwise on V (GpSimd compute measured as a net loss:
    shared SBUF ports stall concurrent V 2x-mode ops ~1:1). GpSimd keeps
    only partition_all_reduce (280ns) for all cross-partition sums.
  - matched-box gather: eq blocks in (t,f) order, PE transposes + 16
    accumulating 128x128x256 fp16 matmuls per 64-f chunk against
    diagonal-mask rhs tiles (TBDG, built on S).
  - binary search 7 iters via partition_all_reduce.
  - cross-image software pipelining: img1's pass-1 groups are emitted
    between img0's tail stages (pass3/epilogue/binary search) so V chews
    intervals while img0's PE/S/DMA serial chains progress. IOU (bufs=1)
    hazard avoided by ordering img0's eq+pass2 before the interleave.
"""

import os
from contextlib import ExitStack

import numpy as np

P = 128          # SBUF partitions
F = 512          # anchors per partition row
T = 32           # targets per image
BI = 2           # images per core
A = P * F        # 65536 anchors
NCORES = 8

EPS = 1e-6
TINY = 1e-38
EPSI = float(2.0 ** -20)          # index-packing epsilon (log space)
DELTA = 2e-6                      # strictness margin for iou > 0.3
LN05 = float(np.log(0.5)) + 15.0 * EPSI
LN04 = float(np.log(0.4)) + 15.0 * EPSI
LN03D = float(np.log(0.3)) + DELTA
N_SEARCH = 5                      # binary-search iterations for kth value
F_T = [float(np.exp((31 - t) * EPSI)) for t in range(T)]


def _emit(nc, tc, ctx, bp_d, cp_d, an_d, tb_d, out_d, sc1_d, sc2_d, sc3_d,
          sc4_d, sc5_d, sc6_d, sc7_d):
    import concourse.bass as bass
    import concourse.bass_isa as bass_isa
    import concourse.mybir as mybir

    f32 = mybir.dt.float32
    f16 = mybir.dt.float16
    Alu = mybir.AluOpType
    Act = mybir.ActivationFunctionType
    Red = bass_isa.ReduceOp
    V = nc.vector
    S = nc.scalar
    G = nc.gpsimd
    PE = nc.tensor
    SY = nc.sync
    ts = bass.ts

    big = ctx.enter_context(tc.tile_pool(name="big", bufs=1))
    stage = ctx.enter_context(tc.tile_pool(name="stage", bufs=1))
    per = ctx.enter_context(tc.tile_pool(name="per", bufs=1))
    tp = ctx.enter_context(tc.tile_pool(name="tp", bufs=2))
    sm = ctx.enter_context(tc.tile_pool(name="sm", bufs=2))
    ps = ctx.enter_context(tc.tile_pool(name="ps", bufs=2, space="PSUM"))

    # ---- input loads first: smalls, then big chunked loads
    TBrow0 = sm.tile([1, T * 4], f32, bufs=2, name="TBrow0")
    SY.dma_start(TBrow0[:], tb_d[0].rearrange("t c -> (t c)")[None, :])
    EV0 = sm.tile([2, 64], f32, bufs=2, name="EV0")
    SY.dma_start(EV0[:].rearrange("p (g c) -> p g c", c=4),
                 tb_d[0].rearrange("(g two) c -> two g c", two=2))
    CPt0 = per.tile([P, F], f32, tag="CPt", bufs=2, name="CPt0")
    SY.dma_start(CPt0[:], cp_d[0].rearrange("(p f) -> p f", p=P))
    AXYS = stage.tile([P, F * 4], f32, tag="BPS")
    anv = an_d.rearrange("(p f) c -> p (f c)", p=P)
    for _q in range(8):
        SY.dma_start(AXYS[:, ts(_q, F // 2)], anv[:, ts(_q, F // 2)])
    BPS0 = stage.tile([P, F * 4], f32, tag="BPS", bufs=1, name="BPS0")
    bpv0 = bp_d[0].rearrange("(p f) c -> p (f c)", p=P)
    for _q in range(8):
        SY.dma_start(BPS0[:, ts(_q, F // 2)], bpv0[:, ts(_q, F // 2)])
    preload = {0: (BPS0, CPt0, TBrow0, EV0)}

    AXv = AXYS[:].rearrange("p (f c) -> p c f", c=4)
    AX1 = big.tile([P, F], f16)
    AY1 = big.tile([P, F], f16)
    AX2 = big.tile([P, F], f16)
    AY2 = big.tile([P, F], f16)

    def anchor_casts():
        for _q in range(8):
            _sl = slice(64 * _q, 64 * (_q + 1))
            V.tensor_copy(AX1[:, _sl], AXv[:, 0, _sl])
            V.tensor_copy(AY1[:, _sl], AXv[:, 1, _sl])
            V.tensor_copy(AX2[:, _sl], AXv[:, 2, _sl])
            V.tensor_copy(AY2[:, _sl], AXv[:, 3, _sl])
    AREAA = big.tile([P, F], f16)

    def areaa_build():
        aw0 = tp.tile([P, F], f16, tag="mx", bufs=3)
        ah0 = tp.tile([P, F], f16, tag="my", bufs=3)
        V.tensor_sub(aw0[:], AX2[:], AX1[:])
        V.tensor_sub(ah0[:], AY2[:], AY1[:])
        V.tensor_mul(AREAA[:], aw0[:], ah0[:])

    # per-target ln(0.3) + (31-t)*EPSI + DELTA on partitions 0..31
    L03row = sm.tile([1, T], f32)
    for t in range(T):
        V.memset(L03row[:, t : t + 1], LN03D + (31 - t) * EPSI)
    SY.dma_start(sc3_d, L03row[:])
    L03C = big.tile([T, 1], f32)
    SY.dma_start(L03C[:], sc3_d.rearrange("(t one) -> t one", one=1))

    ONES = big.tile([P, 1], f32)
    V.memset(ONES[:], 1.0)
    ONESROW = big.tile([1, P], f32)
    V.memset(ONESROW[:], 1.0)
    ONES32 = big.tile([T, P], f32)
    V.memset(ONES32[:], 1.0)
    # identity matrix via DRAM bounce: diag of [128,128] at stride 129
    ZED = stage.tile([P, P], f32, tag="ZED")
    V.memset(ZED[:], 0.0)
    SY.dma_start(sc4_d[: P * P].rearrange("(p f) -> p f", p=P), ZED[:])
    SY.dma_start(
        sc4_d[: P * 129].rearrange("(p f) -> p f", f=129)[:, 0:1], ONES[:])
    IDTV = stage.tile([P, P], f32, tag="IDTV", name="IDTV")
    SY.dma_start(IDTV[:], sc4_d[: P * P].rearrange("(p f) -> p f", p=P))
    IDTH = big.tile([P, P], f16)

    def late_setup():
        # IDTV bounce has long landed by now; no V head-block
        V.tensor_copy(IDTH[:], IDTV[:])

    # DIAG64 [P, 256] f32: D[p, c*64 + f] = (f == p % 64), via DRAM bounce.
    ZED2 = stage.tile([P, 256], f32, tag="ZED2")
    V.memset(ZED2[:], 0.0)
    SY.dma_start(sc5_d[: P * 256].rearrange("(p f) -> p f", p=P), ZED2[:])
    for half in range(2):
        for c in range(4):
            base = half * 64 * 256 + c * 64
            SY.dma_start(
                sc5_d[base : base + 64 * 257]
                .rearrange("(r x) -> r x", x=257)[:, 0:1],
                ONES[0:64, :])
    DIAG64 = big.tile([P, 256], f32)
    SY.dma_start(DIAG64[:], sc5_d[: P * 256].rearrange("(p f) -> p f", p=P))

    # lhsT for the 2-row broadcast matmul (TBALL build), via DRAM bounce
    ZROW = sm.tile([1, 256], f32)
    V.memset(ZROW[:, 0:64], 1.0)
    V.memset(ZROW[:, 64:192], 0.0)
    V.memset(ZROW[:, 192:256], 1.0)
    SY.dma_start(sc6_d, ZROW[:])
    LH2 = big.tile([2, P], f32)
    SY.dma_start(LH2[:], sc6_d.rearrange("(p f) -> p f", p=2))

    ZEDH = stage.tile([P, 256], f16, tag="ZED2", name="ZEDH")
    V.memset(ZEDH[:], 0.0)
    _f7 = sc7_d[: 16 * 32768].rearrange("(p x) -> p x", x=4096)
    for _g in range(16):
        SY.dma_start(_f7[:, 256 * _g : 256 * (_g + 1)], ZEDH[:])
    CTINY = big.tile([P, 1], f32)
    V.memset(CTINY[:], TINY)
    CEPS = big.tile([P, 1], f32)
    V.memset(CEPS[:], EPS)
    OUT = big.tile([1, 2], f32)
    V.memset(OUT[:], 0.0)

    IOU = big.tile([P, T * F], f16)     # packed log-iou (fp16), t-MAJOR

    def iou_t(t):
        return IOU[:, t * F : (t + 1) * F]

    st = [dict() for _ in range(BI)]

    # ================= per-image stages =================

    def prep(b):
        d = st[b]
        if b in preload:
            BPS, CPt, TBrow, EV = preload[b]
        else:
            BPS = stage.tile([P, F * 4], f32, tag="BPS", bufs=1, name="BPS")
            bpv = bp_d[b].rearrange("(p f) c -> p (f c)", p=P)
            for _q in range(4):
                SY.dma_start(BPS[:, ts(_q, F)], bpv[:, ts(_q, F)])
            CPt = per.tile([P, F], f32, tag="CPt", bufs=2, name="CPt")
            SY.dma_start(CPt[:], cp_d[b].rearrange("(p f) -> p f", p=P))
            TBrow = sm.tile([1, T * 4], f32, bufs=2, name="TBrow")
            SY.dma_start(TBrow[:], tb_d[b].rearrange("t c -> (t c)")[None, :])
            EV = sm.tile([2, 64], f32, bufs=2, name="EV")
            SY.dma_start(EV[:].rearrange("p (g c) -> p g c", c=4),
                         tb_d[b].rearrange("(g two) c -> two g c", two=2))
        d["BPS"] = BPS
        d["CPt"] = CPt
        d["EV"] = EV
        TBrowV = sm.tile([1, T * 4], f32, bufs=2, name="TBrowV")
        V.tensor_copy(TBrowV[:], TBrow[:])
        TBB = per.tile([P, T * 4], f32, tag="TBB", bufs=1, name="TBB")
        tbb_ps = ps.tile([P, T * 4], f32, tag="bc_ps", bufs=1, name="tbb_ps")
        PE.matmul(tbb_ps[:], ONESROW[:], TBrowV[:], start=True, stop=True)
        S.copy(TBB[:], tbb_ps[:])
        d["TBB"] = TBB
        NTB = per.tile([P, T * 4], f32, tag="NTB", bufs=1, name="NTB")
        V.tensor_scalar(NTB[:], TBB[:], -1.0, None, op0=Alu.mult)
        d["NTB"] = NTB
        TBv = TBB[:].rearrange("p (t c) -> p c t", c=4)
        TW = per.tile([P, T], f32, tag="TW", bufs=1, name="TW")
        TH = per.tile([P, T], f32, tag="TH", bufs=1, name="TH")
        ABE = per.tile([P, T], f32, tag="ABE", bufs=1, name="ABE")
        V.tensor_sub(TW[:], TBv[:, 2], TBv[:, 0])
        V.tensor_sub(TH[:], TBv[:, 3], TBv[:, 1])
        V.tensor_mul(ABE[:], TW[:], TH[:])
        V.tensor_scalar(ABE[:], ABE[:], EPS, None, op0=Alu.add)
        d["TW"], d["TH"], d["ABE"] = TW, TH, ABE

        d["mp"] = per.tile([P, F], f16, tag="mp", bufs=2, name="mp")
        d["RM8"] = per.tile([P, T * 8], f16, tag="RM8", bufs=1, name="RM8")
        d["groups"] = {}

    def load_bp(b):
        d = st[b]
        BPv = d["BPS"][:].rearrange("p (f c) -> p c f", c=4)
        BX1 = per.tile([P, F], f16, tag="BX1", bufs=1, name="BX1")
        BY1 = per.tile([P, F], f16, tag="BY1", bufs=1, name="BY1")
        BX2 = per.tile([P, F], f16, tag="BX2", bufs=1, name="BX2")
        BY2 = per.tile([P, F], f16, tag="BY2", bufs=1, name="BY2")
        S.copy(BX1[:], BPv[:, 0])
        S.copy(BY1[:], BPv[:, 1])
        S.copy(BX2[:], BPv[:, 2])
        S.copy(BY2[:], BPv[:, 3])
        d["BX1"], d["BY1"], d["BX2"], d["BY2"] = BX1, BY1, BX2, BY2

    def ncol(d, tt, c):
        return d["NTB"][:, 4 * tt + c : 4 * tt + c + 1]

    def pass1_group(b, g):
        d = st[b]
        t0, t1 = 2 * g, 2 * g + 1
        INTER2 = tp.tile([P, 2 * F], f16, tag="INTER2", bufs=3,
                         name=f"INTER2_{b}_{g}")
        UNION2 = tp.tile([P, 2 * F], f16, tag="UNION2", bufs=3,
                         name=f"UNION2_{b}_{g}")
        for k, t in enumerate((t0, t1)):
            # x: dx = min(AX2 - tx1, tw) - relu(AX1 - tx1)   (all fp16)
            rx = tp.tile([P, F], f16, tag="rx", bufs=3, name="rx")
            S.activation(rx[:], AX1[:], Act.Relu, bias=ncol(d, t, 0))
            mx = tp.tile([P, F], f16, tag="mx", bufs=3, name="mx")
            V.tensor_scalar(mx[:], AX2[:], ncol(d, t, 0),
                            d["TW"][:, t : t + 1], op0=Alu.add, op1=Alu.min)
            V.tensor_sub(mx[:], mx[:], rx[:])                    # dx
            # y: dy = min(AY2 - ty1, th) - relu(AY1 - ty1)
            ry = tp.tile([P, F], f16, tag="ry", bufs=3, name="ry")
            S.activation(ry[:], AY1[:], Act.Relu, bias=ncol(d, t, 1))
            my = tp.tile([P, F], f16, tag="my", bufs=3, name="my")
            V.tensor_scalar(my[:], AY2[:], ncol(d, t, 1),
                            d["TH"][:, t : t + 1], op0=Alu.add, op1=Alu.min)
            V.tensor_sub(my[:], my[:], ry[:])                    # dy
            V.tensor_scalar(my[:], my[:], 0.0, F_T[t],
                            op0=Alu.max, op1=Alu.mult)           # dyr (inpl)
            isl = INTER2[:, k * F : (k + 1) * F]
            V.scalar_tensor_tensor(isl, mx[:], 0.0, my[:],
                                   op0=Alu.max, op1=Alu.mult)    # inter
            V.tensor_sub(UNION2[:, k * F : (k + 1) * F], AREAA[:], isl)
        d["groups"][g] = (INTER2, UNION2)

    def packed_group(b, g):
        # emitted one group behind pass1_group so the S queue has the next
        # group's relus ahead of these Lns
        d = st[b]
        t0, t1 = 2 * g, 2 * g + 1
        INTER2, UNION2 = d["groups"].pop(g)
        ABE = d["ABE"]
        LI2 = tp.tile([P, 2 * F], f16, tag="LI2", bufs=3, name="LI2")
        LU2 = tp.tile([P, 2 * F], f16, tag="LU2", bufs=3, name="LU2")
        S.activation(LI2[:], INTER2[:], Act.Ln, bias=CTINY[:])
        S.activation(LU2[:, 0:F], UNION2[:, 0:F], Act.Ln,
                     bias=ABE[:, t0 : t0 + 1])
        S.activation(LU2[:, F : 2 * F], UNION2[:, F : 2 * F], Act.Ln,
                     bias=ABE[:, t1 : t1 + 1])
        V.tensor_tensor(IOU[:, t0 * F : (t1 + 1) * F], LI2[:],
                        LU2[:], op=Alu.subtract)
        V.max(d["RM8"][:, ts(t0, 8)], iou_t(t0))
        V.max(d["RM8"][:, ts(t1, 8)], iou_t(t1))
        mp = d["mp"]
        if g == 0:
            S.copy(mp[:], iou_t(0))
            V.tensor_max(mp[:], mp[:], iou_t(1))
        else:
            V.tensor_max(mp[:], mp[:], iou_t(t0))
            V.tensor_max(mp[:], mp[:], iou_t(t1))

    def thr_a(b):
        d = st[b]
        T8 = d["T8"]
        G8 = sm.tile([T, 8], f16, name="G8")
        V.max(G8[:], T8[:])
        G8F = sm.tile([T, 8], f32, name="G8F")
        S.copy(G8F[:], G8[:])
        mx8 = sm.tile([T, 1], f32, name="mx8")
        V.tensor_max(mx8[:], G8F[:, 2:3], L03C[:])
        thr = sm.tile([T, 1], f32, name="thr")
        V.tensor_tensor(thr[:], mx8[:], G8F[:, 0:1], op=Alu.min)
        # D32 = diag(thr): THR[i, j] = sum_p ones32[p, i] * D32[p, j] = thr[j]
        D32 = sm.tile([T, T], f32, name="D32")
        V.tensor_tensor(D32[:], IDTV[0:T, 0:T], thr[:].broadcast_to([T, T]),
                        op=Alu.mult)
        d["D32"] = D32

    def thr_b(b):
        d = st[b]
        THR = per.tile([P, T], f32, tag="THR", bufs=1, name="THR")
        thr_ps = ps.tile([P, T], f32, tag="bc_ps", bufs=1, name="thr_ps")
        PE.matmul(thr_ps[:], ONES32[:], d["D32"][:], start=True, stop=True)
        S.copy(THR[:], thr_ps[:])
        d["THR"] = THR

    def eq_pass(b):
        d = st[b]
        mp = d["mp"]
        IOU3 = IOU[:].rearrange("p (t f) -> p t f", t=T)
        eqbs = []
        for q in range(8):
            EQB = per.tile([P, T * 64], f16, tag="EQB", bufs=8, name="EQB")
            V.tensor_tensor(
                EQB[:].rearrange("p (t f) -> p t f", t=T),
                IOU3[:, :, q * 64 : (q + 1) * 64],
                mp[:, q * 64 : (q + 1) * 64]
                .rearrange("p (one f) -> p one f", one=1)
                .broadcast_to([P, T, 64]),
                op=Alu.is_equal)
            eqbs.append(EQB)
        d["eqbs"] = eqbs

    def pass2(b):
        d = st[b]
        THR = d["THR"]
        facc = per.tile([P, F], f32, tag="facc", bufs=1, name="facc")
        V.tensor_scalar(facc[:], iou_t(0), THR[:, 0:1], None, op0=Alu.is_ge)
        for t in range(1, T):
            V.scalar_tensor_tensor(facc[:], iou_t(t), THR[:, t : t + 1],
                                   facc[:], op0=Alu.is_ge, op1=Alu.max)
        d["facc"] = facc

    def tbdg_build(b):
        d = st[b]
        # TBALL [P, 64]: row-block h=p//64 holds tb[2g+h, c] at col g*4+c
        EV = d["EV"]
        EVv = sm.tile([2, 64], f32, bufs=2, name="EVv")
        V.tensor_copy(EVv[:], EV[:])
        tball_ps = ps.tile([P, 64], f32, tag="bc_ps", bufs=1, name="tball_ps")
        PE.matmul(tball_ps[:], LH2[:], EVv[:], start=True, stop=True)
        TBALL = per.tile([P, 64], f16, tag="TBALL", bufs=2, name="TBALL")
        S.copy(TBALL[:], tball_ps[:])
        # build TBDG in DRAM (p-major: F[p*4096 + g*256 + j]): the zero
        # background persists; each image rewrites only the diagonals.
        # diag entries for (half, c): offset half*64*4096 + c*64 + r*4097
        # + g*256  -> view [r:64 (4097), g:16 (256)], matching the SBUF
        # source TBALL[64h:64h+64, c::4] ([64 partitions, 16 cols]).
        TBV = TBALL[:].rearrange("p (g c) -> p c g", c=4)
        for half in range(2):
            for c in range(4):
                off = half * 64 * 4096 + c * 64
                SY.dma_start(
                    sc7_d[off : off + 64 * 4097]
                    .rearrange("(r x) -> r x", x=4097)[:, 0:4096]
                    .rearrange("r (g y) -> r g y", y=256)[:, :, 0:1],
                    TBV[64 * half : 64 * half + 64, c, :, None])
        TBDG = per.tile([P, 16 * 256], f16, tag="TBDG", bufs=1, name="TBDG")
        SY.dma_start(TBDG[:], sc7_d[: 16 * 32768].rearrange("(p x) -> p x", x=4096))
        d["TBDG"] = TBDG

    def pass3(b):
        # PE + S only: transposes, gather matmuls, M copies
        d = st[b]
        TBDG = d["TBDG"]
        M0 = per.tile([P, F], f16, tag="M0", bufs=1, name="M0")
        M1 = per.tile([P, F], f16, tag="M1", bufs=1, name="M1")
        M2 = per.tile([P, F], f16, tag="M2", bufs=1, name="M2")
        M3 = per.tile([P, F], f16, tag="M3", bufs=1, name="M3")
        for q in range(8):
            EQB = d["eqbs"][q]
            mm_ps = ps.tile([P, 256], f32, tag="mm_ps", bufs=2, name="mm_ps")
            for h in range(4):
                oh_ps = ps.tile([P, 512], f16, tag="oh_ps", bufs=2,
                                name="oh_ps")
                for j in range(4):
                    PE.transpose(oh_ps[:, ts(j, P)],
                                 EQB[:, ts(4 * h + j, P)], IDTH[:])
                OH = per.tile([P, 512], f16, tag="OH", bufs=2, name="OH")
                S.copy(OH[:], oh_ps[:])
                for j in range(4):
                    g = 4 * h + j
                    PE.matmul(mm_ps[:], OH[:, ts(j, P)],
                              TBDG[:, g * 256 : (g + 1) * 256],
                              start=(g == 0), stop=(g == 15))
            S.copy(M0[:, ts(q, 64)], mm_ps[:, 0:64])
            S.copy(M1[:, ts(q, 64)], mm_ps[:, 64:128])
            S.copy(M2[:, ts(q, 64)], mm_ps[:, 128:192])
            S.copy(M3[:, ts(q, 64)], mm_ps[:, 192:256])
        d["M"] = (M0, M1, M2, M3)

    def epilogue1(b):
        d = st[b]
        mp, facc, CPt = d["mp"], d["facc"], d["CPt"]
        R = [per.tile([P, F], f32, tag=f"R{i}", bufs=1, name=f"R{i}")
             for i in range(4)]
        d["R"] = R
        R0, R1 = R[0][:], R[1][:]
        R2 = R[2][:]
        COLS = per.tile([P, 8], f32, tag="COLS", bufs=1, name="COLS")
        V.memset(COLS[:], 0.0)
        d["COLS"] = COLS

        pos = per.tile([P, F], f32, tag="pos", bufs=1, name="pos")
        V.scalar_tensor_tensor(pos[:], mp[:], LN05, facc[:],
                               op0=Alu.is_ge, op1=Alu.max,
                               accum_out=COLS[:, 0:1])
        neg = per.tile([P, F], f32, tag="neg", bufs=1, name="neg")
        V.scalar_tensor_tensor(R0, mp[:], LN04, facc[:],
                               op0=Alu.is_lt, op1=Alu.subtract)
        V.tensor_scalar(neg[:], R0, 0.0, None, op0=Alu.max, op1=Alu.add,
                        accum_out=COLS[:, 1:2])

        # focal: fl = (0.5*pos - 0.75) * (pt-1)^2 * ln(pt)
        fl = per.tile([P, F], f32, tag="fl", bufs=1, name="fl")
        V.tensor_mul(R0, pos[:], CPt[:])
        V.scalar_tensor_tensor(R0, R0, 2.0, CPt[:],
                               op0=Alu.mult, op1=Alu.subtract)
        V.tensor_sub(R0, R0, pos[:])                 # pt - 1
        S.activation(R1, R0, Act.Ln, bias=1.0)       # ln(pt)
        S.activation(R2, R0, Act.Square)             # (1-pt)^2
        V.tensor_mul(R1, R1, R2)
        V.tensor_scalar(R0, pos[:], 0.5, 0.75, op0=Alu.mult, op1=Alu.subtract)
        V.tensor_mul(fl[:], R0, R1)
        V.scalar_tensor_tensor(R0, fl[:], 1.0, pos[:],
                               op0=Alu.mult, op1=Alu.mult,
                               accum_out=COLS[:, 2:3])
        ns = per.tile([P, F], f32, tag="ns", bufs=1, name="ns")
        V.tensor_mul(ns[:], fl[:], neg[:])
        d["pos"], d["ns"], d["fl"] = pos, ns, fl

    def epilogue2(b):
        d = st[b]
        M0, M1, M2, M3 = d["M"]
        BX1, BY1, BX2, BY2 = d["BX1"][:], d["BY1"][:], d["BX2"][:], d["BY2"][:]
        E = [per.tile([P, F], f16, tag=f"E{i}", bufs=1, name=f"E{i}")
             for i in range(5)]
        d["E"] = E
        R0, R1, R2, R3, R4 = [e[:] for e in E]
        V.tensor_max(R0, BX1, M0[:])
        V.tensor_tensor(R1, BX2, M2[:], op=Alu.min)
        V.tensor_sub(R1, R1, R0)
        V.tensor_scalar(R1, R1, 0.0, None, op0=Alu.max)   # dxc
        V.tensor_max(R0, BY1, M1[:])
        V.tensor_tensor(R2, BY2, M3[:], op=Alu.min)
        V.tensor_sub(R2, R2, R0)
        V.tensor_scalar(R2, R2, 0.0, None, op0=Alu.max)   # dyc
        V.tensor_mul(R1, R1, R2)                          # bint
        V.tensor_sub(R0, BX2, BX1)
        V.tensor_sub(R2, BY2, BY1)
        V.tensor_mul(R0, R0, R2)                          # areab
        V.tensor_sub(R2, M2[:], M0[:])
        V.tensor_sub(R3, M3[:], M1[:])
        V.tensor_mul(R2, R2, R3)                          # aream
        V.tensor_add(R0, R0, R2)
        V.tensor_sub(R0, R0, R1)                          # uni
        S.activation(R2, R0, Act.Ln, bias=CEPS[:])
        S.activation(R2, R2, Act.Exp, scale=-1.0)         # 1/(uni+eps)
        V.tensor_mul(R1, R1, R2)                          # ioub
        V.tensor_max(R2, BX2, M2[:])
        V.tensor_tensor(R3, BX1, M0[:], op=Alu.min)
        V.tensor_sub(R2, R2, R3)                          # dex
        V.tensor_max(R3, BY2, M3[:])
        V.tensor_tensor(R4, BY1, M1[:], op=Alu.min)
        V.tensor_sub(R3, R3, R4)                          # dey
        V.tensor_mul(R2, R2, R3)                          # enc
        S.activation(R3, R2, Act.Ln, bias=CEPS[:])
        S.activation(R3, R3, Act.Exp, scale=-1.0)         # 1/(enc+eps)
        V.tensor_sub(R2, R2, R0)                          # enc-uni
        V.tensor_mul(R2, R2, R3)
        V.tensor_sub(R2, R2, R1)                  # pb0 = (enc-uni)/(enc+e)-iou

    def epilogue3(b):
        d = st[b]
        M0, M1, M2, M3 = d["M"]
        BX1, BY1, BX2, BY2 = d["BX1"][:], d["BY1"][:], d["BX2"][:], d["BY2"][:]
        E = d["E"]
        R0, R1, R2, R3 = [e[:] for e in E[:4]]
        COLS, pos = d["COLS"], d["pos"]
        V.tensor_sub(R0, BX1, M0[:])
        S.activation(R0, R0, Act.Abs)
        V.tensor_sub(R1, BY1, M1[:])
        S.activation(R1, R1, Act.Abs)
        V.tensor_add(R0, R0, R1)
        V.tensor_sub(R1, BX2, M2[:])
        S.activation(R1, R1, Act.Abs)
        V.tensor_sub(R3, BY2, M3[:])
        S.activation(R3, R3, Act.Abs)
        V.tensor_add(R1, R1, R3)
        V.tensor_add(R0, R0, R1)                          # l1 sum
        V.scalar_tensor_tensor(R2, R0, 0.125, R2, op0=Alu.mult, op1=Alu.add)
        V.scalar_tensor_tensor(R3, R2, 1.0, pos[:],
                               op0=Alu.add, op1=Alu.mult,
                               accum_out=COLS[:, 3:4])

    def sums_kk(b):
        # only needs COLS[:, 0:2] (npos/nneg), ready right after epilogue1 -
        # lets the binary search interleave with the giou block
        d = st[b]
        SUMS = sm.tile([P, 2], f32, name="SUMS")
        G.partition_all_reduce(SUMS[:], d["COLS"][:, 0:2], P, Red.add)
        kk = per.tile([P, 1], f32, tag="kk", bufs=1, name="kk")
        npos = SUMS[:, 0:1]
        nneg = SUMS[:, 1:2]
        V.scalar_tensor_tensor(kk[:], npos, 3.0, nneg, op0=Alu.mult,
                               op1=Alu.min)
        k0 = sm.tile([P, 1], f32, name="k0")
        V.tensor_scalar(k0[:], nneg, 100.0, None, op0=Alu.min)
        zz = sm.tile([P, 1], f32, name="zz")
        V.tensor_scalar(zz[:], npos, 0.5, None, op0=Alu.is_lt)
        kd = sm.tile([P, 1], f32, name="kd")
        V.tensor_sub(kd[:], k0[:], kk[:])
        V.tensor_mul(kd[:], kd[:], zz[:])
        V.tensor_add(kk[:], kk[:], kd[:])
        d["kk"], d["SUMS"] = kk, SUMS
        lo = per.tile([P, 1], f32, tag="lo", bufs=1, name="lo")
        V.memset(lo[:], 0.0)
        d["lo"] = lo
        d["step"] = 2.0

    def bins_iter(b, it):
        d = st[b]
        ns, kk, lo = d["ns"], d["kk"], d["lo"]
        R5 = d["R"][0][:]
        step = d["step"]
        tau = sm.tile([P, 1], f32, name="tau")
        V.tensor_scalar(tau[:], lo[:], step, None, op0=Alu.add)
        cntc = sm.tile([P, 1], f32, name="cntc")
        V.tensor_scalar(R5, ns[:], tau[:], None, op0=Alu.is_gt,
                        op1=Alu.add, accum_out=cntc[:])
        cntb = sm.tile([P, 1], f32, name="cntb")
        G.partition_all_reduce(cntb[:], cntc[:], P, Red.add)
        gg = sm.tile([P, 1], f32, name="gg")
        V.tensor_tensor(gg[:], cntb[:], kk[:], op=Alu.is_ge)
        V.scalar_tensor_tensor(lo[:], gg[:], step, lo[:],
                               op0=Alu.mult, op1=Alu.add)
        d["step"] = step * 0.5

    def bins_final(b):
        d = st[b]
        ns, kk, lo, SUMS = d["ns"], d["kk"], d["lo"], d["SUMS"]
        R5 = d["R"][0][:]
        npos = SUMS[:, 0:1]
        SUMS34 = sm.tile([P, 2], f32, name="SUMS34")
        G.partition_all_reduce(SUMS34[:], d["COLS"][:, 2:4], P, Red.add)
        pfs = SUMS34[:, 0:1]
        pbs = SUMS34[:, 1:2]
        cnt2 = sm.tile([P, 2], f32, name="cnt2")
        V.tensor_scalar(R5, ns[:], lo[:], None, op0=Alu.is_gt,
                        op1=Alu.add, accum_out=cnt2[:, 0:1])
        V.scalar_tensor_tensor(R5, ns[:], lo[:], ns[:],
                               op0=Alu.is_gt, op1=Alu.mult,
                               accum_out=cnt2[:, 1:2])
        C2 = sm.tile([P, 2], f32, name="C2")
        G.partition_all_reduce(C2[:], cnt2[:], P, Red.add)
        nf = C2[:, 0:1]
        sf = C2[:, 1:2]

        kmn = sm.tile([P, 1], f32, name="kmn")
        V.tensor_sub(kmn[:], kk[:], nf)
        V.tensor_mul(kmn[:], kmn[:], lo[:])
        cnum = sm.tile([P, 1], f32, name="cnum")
        V.tensor_add(cnum[:], pfs, sf)
        V.tensor_add(cnum[:], cnum[:], kmn[:])
        den = sm.tile([P, 1], f32, name="den")
        V.tensor_add(den[:], npos, kk[:])
        V.tensor_scalar(den[:], den[:], 1.0, None, op0=Alu.max)
        rden = sm.tile([P, 1], f32, name="rden")
        V.reciprocal(rden[:], den[:])
        confl = sm.tile([P, 1], f32, name="confl")
        V.tensor_mul(confl[:], cnum[:], rden[:])

        np1 = sm.tile([P, 1], f32, name="np1")
        V.tensor_scalar(np1[:], npos, 1.0, None, op0=Alu.max)
        rnp = sm.tile([P, 1], f32, name="rnp")
        V.reciprocal(rnp[:], np1[:])
        bl0 = sm.tile([P, 1], f32, name="bl0")
        V.tensor_mul(bl0[:], pbs, rnp[:])
        zp = sm.tile([P, 1], f32, name="zp")
        V.tensor_scalar(zp[:], npos, 0.5, None, op0=Alu.is_gt)
        bbl = sm.tile([P, 1], f32, name="bbl")
        V.tensor_mul(bbl[:], bl0[:], zp[:])

        V.tensor_add(OUT[0:1, 0:1], OUT[0:1, 0:1], confl[0:1, :])
        V.tensor_add(OUT[0:1, 1:2], OUT[0:1, 1:2], bbl[0:1, :])

    def pass1_all(b, tail_chunks=None):
        # pass1 groups; optional other-image tail chunks interleaved
        tail_chunks = list(tail_chunks or [])
        NG = T // 2
        ci = 0
        for g in range(NG):
            pass1_group(b, g)
            if g >= 1:
                packed_group(b, g - 1)
            if g >= 2 and ci < len(tail_chunks):
                tail_chunks[ci]()
                ci += 1
            if g == 8:
                # first half of RM8 (t=0..15) is final; start its bounce
                SY.dma_start(
                    sc1_d[b][: T * P * 4].rearrange(
                        "(t p j) -> p t j", p=P, t=T // 2),
                    st[b]["RM8"][:, : T * 4].rearrange(
                        "p (t j) -> p t j", t=T // 2))
            if g == 9:
                T8 = stage.tile([T, P * 8], f16, tag="BPS", bufs=1, name="T8")
                st[b]["T8"] = T8
                SY.dma_start(T8[0 : T // 2, :],
                             sc1_d[b][: T * P * 4].rearrange(
                                 "(t x) -> t x", t=T // 2))
        packed_group(b, NG - 1)
        SY.dma_start(
            sc1_d[b][T * P * 4 :].rearrange(
                "(t p j) -> p t j", p=P, t=T // 2),
            st[b]["RM8"][:, T * 4 :].rearrange(
                "p (t j) -> p t j", t=T // 2))
        SY.dma_start(st[b]["T8"][T // 2 : T, :],
                     sc1_d[b][T * P * 4 :].rearrange(
                         "(t x) -> t x", t=T // 2))
        while ci < len(tail_chunks):
            tail_chunks[ci]()
            ci += 1

    def tail_stages(b):
        bi = [(lambda it=it: bins_iter(b, it)) for it in range(N_SEARCH)]
        chunks = [lambda: epilogue1(b), lambda: sums_kk(b),
                  lambda: epilogue2(b), bi[0], bi[1],
                  lambda: epilogue3(b)] + bi[2:]
        chunks += [lambda: bins_final(b)]
        return chunks

    # ================= schedule =================
    prep(0)
    anchor_casts()
    areaa_build()
    load_bp(0)
    pass1_all(0)
    late_setup()
    eq_pass(0)
    thr_a(0)
    tbdg_build(0)   # fills the sc2 bounce round trip
    thr_b(0)
    pass2(0)
    pass3(0)
    prep(1)
    # img1 pass1 interleaved with img0 tail; load_bp(1) overwrites the BX/BY
    # tiles img0's epilogue reads, so it must follow epilogue3(0)
    t0c = tail_stages(0)
    chunks = t0c[:3] + [lambda: load_bp(1)] + t0c[3:]
    during, after = chunks[:-3], chunks[-3:]
    pass1_all(1, tail_chunks=during)
    eq_pass(1)
    thr_a(1)
    after[0]()
    tbdg_build(1)
    after[1]()
    thr_b(1)
    after[2]()
    pass2(1)
    pass3(1)
    for c in tail_stages(1):
        c()

    SY.dma_start(out_d, OUT[0:1, :])


def build():
    import concourse.bacc as bacc
    import concourse.mybir as mybir
    import concourse.tile as tile

    f32 = mybir.dt.float32
    nc = bacc.Bacc("TRN2", target_bir_lowering=False, debug=False)
    bp_d = nc.dram_tensor("bp", [BI, A, 4], f32, kind="ExternalInput")
    cp_d = nc.dram_tensor("cp", [BI, A], f32, kind="ExternalInput")
    an_d = nc.dram_tensor("an", [A, 4], f32, kind="ExternalInput")
    tb_d = nc.dram_tensor("tb", [BI, T, 4], f32, kind="ExternalInput")
    out_d = nc.dram_tensor("out", [2], f32, kind="ExternalOutput")
    sc1_d = nc.dram_tensor("scratch1", [BI, T * P * 8], mybir.dt.float16)
    sc2_d = nc.dram_tensor("scratch2", [BI, T], f32)
    sc3_d = nc.dram_tensor("scratch3", [T], f32)
    sc4_d = nc.dram_tensor("scratch4", [P * 129], f32)
    sc5_d = nc.dram_tensor("scratch5", [P * 256 + 256], f32)
    sc6_d = nc.dram_tensor("scratch6", [256], f32)
    sc7_d = nc.dram_tensor("scratch7", [16 * 32768 + 256], mybir.dt.float16)
    with tile.TileContext(nc) as tc:
        with ExitStack() as ctx:
            _emit(nc, tc, ctx, bp_d.ap(), cp_d.ap(), an_d.ap(), tb_d.ap(),
                  out_d.ap(), sc1_d.ap(), sc2_d.ap(), sc3_d.ap(), sc4_d.ap(),
                  sc5_d.ap(), sc6_d.ap(), sc7_d.ap())
    nc.compile()
    return nc


def kernel(bbox_pred, conf_pred, anchors, target_boxes):
    from concourse.bass_utils import run_bass_kernel_spmd

    bp = np.ascontiguousarray(np.asarray(bbox_pred, dtype=np.float32))
    cp = np.ascontiguousarray(np.asarray(conf_pred, dtype=np.float32))
    an = np.ascontiguousarray(np.asarray(anchors, dtype=np.float32))
    tb = np.ascontiguousarray(np.asarray(target_boxes, dtype=np.float32))

    nc = build()
    in_maps = []
    for k in range(NCORES):
        sl = slice(BI * k, BI * (k + 1))
        in_maps.append({"bp": bp[sl], "cp": cp[sl], "an": an, "tb": tb[sl]})
    trace = bool(int(os.environ.get("DETLOSS_TRACE", "0")))
    res = run_bass_kernel_spmd(nc, in_maps, list(range(NCORES)), trace=trace)
    partials = np.stack([res.results[k]["out"] for k in range(NCORES)])  # [8,2]
    conf = np.float32(partials[:, 0].sum() / 16.0)
    bbox = np.float32(partials[:, 1].sum() / 16.0)
    total = np.float32(conf + bbox)
    if trace:
        kernel.last_exec_time_ns = res.exec_time_ns
        kernel.last_trace = res.instructions_and_trace
    return (total, conf, bbox)
